# revision 16
# baseline (speedup 1.0000x reference)
"""Trainium2 Bass kernel for nn_Model_14998025797662 (Mamba-TimeVariant classifier).

Self-contained: hardcodes shapes. Data-parallel over batch: 16 samples ->
8 cores x 2 samples. Layout: channels-on-partitions, time-on-free.

v2: both per-core samples are packed along the free axis (T2 = 2048) so every
elementwise/scan op covers both samples in one instruction. The scan resets at
the sample boundary via a poisoned dt column (dt=+30 -> dA=exp(A*30)=0).
Engine budget: SSM scans pinned on DVE; dA exponentials on ACT; dBu/C-term
muls split DVE(bf16 2x)/Pool; the 60 state-accumulate adds run as SWDGE
accumulate-DMAs (free of engine time). B/C broadcasts via DRAM-row bounce.
Head transposes via xbar DMA-transpose. LayerNorm stats batched across the 16
time chunks to kill the serial scalar chain.
"""

import numpy as np

import concourse.bacc as bacc
import concourse.bass as bass
from concourse import mybir
from concourse.bass import ds, ts
from concourse.tile import TileContext

F32 = mybir.dt.float32
BF16 = mybir.dt.bfloat16
AF = mybir.ActivationFunctionType
OP = mybir.AluOpType
AX = mybir.AxisListType

B, L, CIN = 16, 1024, 12
DM, DS, DC, DI, DTR = 256, 16, 4, 512, 16
NCLS, NH, EK = 10, 8, 3
NCORES = 8
BLOC = B // NCORES          # 2 samples per core
T2 = BLOC * L               # 2048 combined free axis
NDT = DI // 128             # 4 d-tiles
NCH = T2 // 128             # 16 time chunks
KD = EK * CIN               # 36
PAD = DC - 1                # 3 pad cols per sample for the causal conv
EW = T2 + BLOC * PAD        # 2054 emb width


def _off(n):
    """emb col offset of 512-chunk n (pads at [0:3] and [1027:1030])."""
    return PAD + n * 512 + PAD * (n >= 2)


def _pool_dbu(s, d):
    # dBu feeds the DVE scan directly; keeping it on DVE avoids scan stalls
    return False


def _pool_term(s, d):
    # C-side terms are off the critical path (they feed SWDGE accum DMAs)
    return not (d == (s % 4) and s % 2 == 0)


def _patch_act_tables():
    """Bias ACT table-set selection so Exp and Ln resolve to the same set
    (avoids per-op table thrash). Idempotent."""
    import concourse.bacc as _bacc
    import concourse.hw_specs as _hw
    if getattr(_bacc, "_ant_act_tables_patched", False):
        return
    _orig = _hw.get_activation_tables

    def patched(arch):
        t = _orig(arch)
        both = None
        for name, fns in t.items():
            sn = {str(x).split(".")[-1] for x in fns}
            if "Exp" in sn and "Ln" in sn:
                both = name
                break
        if both is not None:
            for name, fns in t.items():
                if name == both:
                    continue
                fns.discard(mybir.ActivationFunctionType.Exp)
                fns.discard(mybir.ActivationFunctionType.Ln)
        return t

    _bacc.get_activation_tables = patched
    _bacc._ant_act_tables_patched = True


def _build_module():
    _patch_act_tables()
    nc = bacc.Bacc("TRN2", target_bir_lowering=False)

    def din(name, shape, dt=F32):
        return nc.dram_tensor(name, shape, dt, kind="ExternalInput")

    xT = din("xT", [BLOC, CIN, L], BF16)
    xmark2 = din("xmark2", [BLOC, L], BF16)
    tok_lhsT = din("tok_lhsT", [KD, DM], BF16)
    peT = din("peT", [DM, L])
    inWzT = din("inWzT", [DM, DI], BF16)    # z half of in_proj
    convWT = din("convWT", [DC * DM, DI], BF16)
    dcb = din("dcb", [DI, 1])
    xprojWT = din("xprojWT", [DI, DTR + 2 * DS], BF16)
    dtWT = din("dtWT", [DTR, DI], BF16)
    dtb = din("dtb", [DI, 1])
    Amat = din("Amat", [DI, DS])
    Dv = din("Dv", [DI, 1])
    WoutT = din("WoutT", [DI, DM], BF16)
    lng_bc = din("lng_bc", [128, DM], BF16)
    lnb_bc = din("lnb_bc", [128, DM])
    headWT = din("headWT", [DM, NCLS + NH], BF16)
    biasrow = din("biasrow", [1, NCLS + NH], BF16)
    onesrow = din("onesrow", [1, 128], BF16)
    onec = din("onec", [128, 1])
    epsc = din("epsc", [128, 1])

    out = nc.dram_tensor("out", [BLOC, NCLS], F32, kind="ExternalOutput")
    scr_bc = nc.dram_tensor("scr_bc", [2 * DS, T2], BF16)   # rows 0:16 C, 16:32 B
    scr_am = nc.dram_tensor("scr_am", [BLOC, L], F32)
    scr_wx = nc.dram_tensor("scr_wx", [BLOC, L], F32)

    with TileContext(nc) as tc:
        with (
            tc.tile_pool(name="const", bufs=1) as cp,
            tc.tile_pool(name="persist", bufs=1) as pp,
            tc.tile_pool(name="work", bufs=2) as wp,
            tc.tile_pool(name="small", bufs=2) as sp,
            tc.tile_pool(name="psumr", bufs=3, space="PSUM") as psr,
        ):
            def cload(name, shape, src, dt=F32):
                t = cp.tile(shape, dt, name=f"c_{name}")
                nc.sync.dma_start(t[:], src)
                return t

            # inputs + first-use weights load first so stage A starts early
            tokW_sb = cload("tokW", [KD, DM], tok_lhsT[:], BF16)
            rhs36 = pp.tile([KD, T2], BF16, name="rhs36", tag="skinny", bufs=2)
            for b in range(BLOC):
                c0 = b * L
                nc.sync.dma_start(rhs36[12:24, c0:c0 + L], xT[b, :, :])
                nc.sync.dma_start(rhs36[0:12, c0 + 1:c0 + L], xT[b, :, 0:L - 1])
                nc.sync.dma_start(rhs36[0:12, c0:c0 + 1], xT[b, :, 0:1])
                nc.sync.dma_start(rhs36[24:36, c0:c0 + L - 1], xT[b, :, 1:L])
                nc.sync.dma_start(rhs36[24:36, c0 + L - 1:c0 + L], xT[b, :, L - 1:L])
            pe_sb = []
            for m in range(2):
                # staged in the dbu work-tag ring (dead before stage B uses it)
                pt = wp.tile([128, L], F32, name=f"pe{m}", tag="dbu", bufs=2)
                nc.sync.dma_start(pt[:], peT[ts(m, 128), :])
                pe_sb.append(pt)
            inWz_sb = [cload(f"inWz{k}", [128, DI], inWzT[ts(k, 128), :], BF16) for k in range(2)]
            convW_sb = [cload(f"cvW{k}", [128, DI], convWT[ts(k, 128), :], BF16) for k in range(8)]
            dcb_sb = [cload(f"dcb{d}", [128, 1], dcb[ts(d, 128), :]) for d in range(NDT)]
            xprojW_sb = [cload(f"xpW{d}", [128, DTR + 2 * DS], xprojWT[ts(d, 128), :], BF16) for d in range(NDT)]
            dtW_sb = cload("dtW", [DTR, DI], dtWT[:], BF16)
            dtb_sb = [cload(f"dtb{d}", [128, 1], dtb[ts(d, 128), :]) for d in range(NDT)]
            A_sb = [cload(f"A{d}", [128, DS], Amat[ts(d, 128), :]) for d in range(NDT)]
            Dv_sb = [cload(f"Dv{d}", [128, 1], Dv[ts(d, 128), :]) for d in range(NDT)]
            Wout_sb = [cload(f"Wo{d}", [128, DM], WoutT[ts(d, 128), :], BF16) for d in range(NDT)]
            lng_sb = cload("lng", [128, DM], lng_bc[:], BF16)
            lnb_sb = cload("lnb", [128, DM], lnb_bc[:])
            headW_sb = [cload(f"hW{k}", [128, NCLS + NH], headWT[ts(k, 128), :], BF16) for k in range(2)]
            bias_sb = cload("biasrow", [1, NCLS + NH], biasrow[:], BF16)
            ones_sb = cload("onesrow", [1, 128], onesrow[:], BF16)
            one_sb = cload("onec", [128, 1], onec[:])
            eps_sb = cload("epsc", [128, 1], epsc[:])
            xmrow = wp.tile([BLOC, L], BF16, name="xmrow", tag="xmk", bufs=1)
            nc.sync.dma_start(xmrow[:], xmark2[:, :])

            # ======== stage A ========

            emb_sb = [pp.tile([128, EW], BF16, name=f"emb{m}", tag="embh", bufs=3) for m in range(2)]
            for m in range(2):
                nc.vector.memset(emb_sb[m][:, 0:PAD], 0.0)
                nc.vector.memset(emb_sb[m][:, PAD + L:PAD + L + PAD], 0.0)

            sz_sb = [pp.tile([128, T2], BF16, name=f"sz{d}", tag=f"sz{d}") for d in range(NDT)]
            u0_sb = [pp.tile([128, T2], BF16, name=f"u0{d}", tag=f"u0{d}") for d in range(NDT)]
            xdbl_sb = pp.tile([DTR + 2 * DS, T2], BF16, name="xdbl", tag="skinny", bufs=2)
            dt_sb = [pp.tile([128, T2], BF16, name=f"dt{d}", tag=f"dt{d}") for d in range(NDT)]

            # pass 1 (silu act-table): emb, z-silu, conv-silu per chunk
            for n in range(4):
                o = _off(n)
                # emb chunk: tok conv + positional embedding
                for m in range(2):
                    ps = psr.tile([128, 512], F32, name=f"eps{m}{n}", tag="ps512")
                    nc.tensor.matmul(ps[:], tokW_sb[:, ts(m, 128)], rhs36[:, ts(n, 512)],
                                     start=True, stop=True)
                    nc.vector.tensor_add(emb_sb[m][:, ds(o, 512)], ps[:],
                                         pe_sb[m][:, ds((n % 2) * 512, 512)])
                # z half -> silu
                for d in range(NDT):
                    ps = psr.tile([128, 512], F32, name=f"z{d}{n}", tag="ps512")
                    for k in range(2):
                        nc.tensor.matmul(ps[:], inWz_sb[k][:, ts(d, 128)],
                                         emb_sb[k][:, ds(o, 512)],
                                         start=(k == 0), stop=(k == 1))
                    nc.scalar.activation(sz_sb[d][:, ts(n, 512)], ps[:], AF.Silu)
                # fused causal conv of in_proj x-half -> silu
                for d in range(NDT):
                    ps = psr.tile([128, 512], F32, name=f"u{d}{n}", tag="ps512")
                    for k in range(8):
                        j = k // 2
                        nc.tensor.matmul(ps[:], convW_sb[k][:, ts(d, 128)],
                                         emb_sb[k % 2][:, ds(o - PAD + j, 512)],
                                         start=(k == 0), stop=(k == 7))
                    nc.scalar.activation(u0_sb[d][:, ts(n, 512)], ps[:], AF.Silu,
                                         bias=dcb_sb[d][:, 0:1])
            # pass 2 (no act table): x_proj -> x_dblT
            for n in range(4):
                ps = psr.tile([DTR + 2 * DS, 512], F32, name=f"xd{n}", tag="ps512")
                for k in range(NDT):
                    nc.tensor.matmul(ps[:], xprojW_sb[k][:], u0_sb[k][:, ts(n, 512)],
                                     start=(k == 0), stop=(k == NDT - 1))
                nc.scalar.copy(xdbl_sb[:, ts(n, 512)], ps[:])
            # pass 3 (exp/ln act-table): dt = softplus
            for n in range(4):
                for d in range(NDT):
                    ps = psr.tile([128, 512], F32, name=f"dtp{d}{n}", tag="ps512")
                    nc.tensor.matmul(ps[:], dtW_sb[:, ts(d, 128)], xdbl_sb[0:DTR, ts(n, 512)],
                                     start=True, stop=True)
                    esp = psr.tile([128, 512], F32, name=f"esp{d}{n}", tag="ps512")
                    nc.scalar.activation(esp[:], ps[:], AF.Exp, bias=dtb_sb[d][:, 0:1])
                    nc.scalar.activation(dt_sb[d][:, ts(n, 512)], esp[:], AF.Ln,
                                         bias=one_sb[:, 0:1])

            # stage B/C broadcast rows to DRAM (single wide write)
            nc.sync.dma_start(scr_bc[:, :], xdbl_sb[DTR:DTR + 2 * DS, :])

            # w = dt*u, then poison dt col at the sample boundary so dA goes
            # to 0 there (scan state reset)
            wT_sb = [pp.tile([128, T2], BF16, name=f"w{d}", tag=f"w{d}") for d in range(NDT)]
            for d in range(NDT):
                nc.vector.tensor_mul(wT_sb[d][:], dt_sb[d][:], u0_sb[d][:])
            for d in range(NDT):
                nc.vector.memset(dt_sb[d][:, L:L + 1], 30.0)

            # ======== stage B: 16 SSM states ========
            acc = [pp.tile([128, T2], BF16, name=f"acc{d}", tag=f"acc{d}") for d in range(NDT)]
            for s in range(DS):
                bbc = wp.tile([128, T2], BF16, name=f"bbc{s}", tag="bbc", bufs=2)
                nc.sync.dma_start(bbc[:], scr_bc[DS + s:DS + s + 1, :].to_broadcast((128, T2)))
                cbc = wp.tile([128, T2], BF16, name=f"cbc{s}", tag="cbc", bufs=2)
                nc.sync.dma_start(cbc[:], scr_bc[s:s + 1, :].to_broadcast((128, T2)))
                dAs, dBus, hs = [], [], []
                for d in range(NDT):
                    dA = wp.tile([128, T2], F32, name=f"dA{s}{d}", tag="dA", bufs=3)
                    nc.scalar.activation(dA[:], dt_sb[d][:], AF.Exp, scale=A_sb[d][:, s:s + 1])
                    dAs.append(dA)
                for d in range(NDT):
                    dBu = wp.tile([128, T2], BF16, name=f"dBu{s}{d}", tag="dbu", bufs=2)
                    eng = nc.gpsimd if _pool_dbu(s, d) else nc.vector
                    eng.tensor_mul(dBu[:], wT_sb[d][:], bbc[:])
                    dBus.append(dBu)
                for d in range(NDT):
                    h = wp.tile([128, T2], BF16, name=f"h{s}{d}", tag="embh", bufs=3)
                    nc.vector.tensor_tensor_scan(h[:], dAs[d][:], dBus[d][:], 0.0,
                                                 op0=OP.mult, op1=OP.add)
                    hs.append(h)
                for d in range(NDT):
                    eng = nc.gpsimd if _pool_term(s, d) else nc.vector
                    if s == 0:
                        eng.tensor_mul(acc[d][:], hs[d][:], cbc[:])
                    else:
                        term = wp.tile([128, T2], BF16, name=f"term{s}{d}", tag="term", bufs=2)
                        eng.tensor_mul(term[:], hs[d][:], cbc[:])
                        nc.gpsimd.dma_start(acc[d][:], term[:], accum_op=OP.add)

            # ytot = (acc + u*D) * sz, stored back into the w tiles
            for d in range(NDT):
                t1 = wp.tile([128, T2], BF16, name=f"yt1{d}", tag="dbu", bufs=2)
                nc.vector.scalar_tensor_tensor(t1[:], u0_sb[d][:], Dv_sb[d][:, 0:1],
                                               acc[d][:], op0=OP.mult, op1=OP.add)
                eng = nc.gpsimd if d % 2 else nc.vector
                eng.tensor_mul(wT_sb[d][:], t1[:], sz_sb[d][:])

            # ======== stage C ========
            mo_ps, mosb = [], []
            ssum_all = sp.tile([128, NCH], F32, name="ssum_all", tag="ssum", bufs=1)
            sqs_all = sp.tile([128, NCH], F32, name="sqs_all", tag="sqs", bufs=1)
            for t in range(NCH):
                mp = psr.tile([128, DM], F32, name=f"mo{t}", tag="MO", bufs=2)
                for d in range(NDT):
                    nc.tensor.matmul(mp[:], wT_sb[d][:, ts(t, 128)], Wout_sb[d][:],
                                     start=(d == 0), stop=(d == NDT - 1))
                ms = pp.tile([128, DM], BF16, name=f"mosb{t}", tag=f"mos{t}")
                nc.scalar.activation(ms[:], mp[:], AF.Identity,
                                     accum_out=ssum_all[:, t:t + 1])
                sq = wp.tile([128, DM], BF16, name=f"sq{t}", tag="sqo", bufs=2)
                nc.scalar.activation(sq[:], ms[:], AF.Square,
                                     accum_out=sqs_all[:, t:t + 1])
                mosb.append(ms)

            # batched LN stats over all 16 chunks
            mun = sp.tile([128, NCH], F32, name="mun", tag="mun")
            nc.vector.tensor_scalar_mul(mun[:], ssum_all[:], -1.0 / DM)
            m2t = sp.tile([128, NCH], F32, name="m2t", tag="m2t")
            nc.vector.tensor_scalar_mul(m2t[:], sqs_all[:], 1.0 / DM)
            msq = sp.tile([128, NCH], F32, name="msq", tag="msq")
            nc.vector.tensor_mul(msq[:], mun[:], mun[:])
            var = sp.tile([128, NCH], F32, name="var", tag="var")
            nc.vector.tensor_sub(var[:], m2t[:], msq[:])
            lnv = sp.tile([128, NCH], F32, name="lnv", tag="lnv")
            nc.scalar.activation(lnv[:], var[:], AF.Ln, bias=eps_sb[:, 0:1])
            rstd = sp.tile([128, NCH], F32, name="rstd", tag="rstd")
            nc.scalar.activation(rstd[:], lnv[:], AF.Exp, scale=-0.5)
            nmr = sp.tile([128, NCH], F32, name="nmr", tag="nmr")
            nc.vector.tensor_mul(nmr[:], mun[:], rstd[:])

            lg_all = []
            for t in range(NCH):
                xn = wp.tile([128, DM], F32, name=f"xn{t}", tag="xn", bufs=2)
                nc.scalar.activation(xn[:], mosb[t][:], AF.Identity,
                                     bias=nmr[:, t:t + 1], scale=rstd[:, t:t + 1])
                t1 = wp.tile([128, DM], F32, name=f"t1{t}", tag="t1", bufs=2)
                nc.gpsimd.tensor_mul(t1[:], xn[:], lng_sb[:])
                t2 = wp.tile([128, DM], F32, name=f"t2{t}", tag="t2", bufs=2)
                nc.gpsimd.tensor_add(t2[:], t1[:], lnb_sb[:])
                mam = wp.tile([128, DM], BF16, name=f"mam{t}", tag="mam", bufs=3)
                nc.scalar.activation(mam[:], t2[:], AF.Silu)
                moT = [wp.tile([128, 128], BF16, name=f"moT{t}{m}", tag="moT", bufs=3)
                       for m in range(2)]
                for m in range(2):
                    nc.sync.dma_start(moT[m][:], mam[:, ts(m, 128)], transpose=True)
                hd = psr.tile([128, NCLS + NH], F32, name=f"hd{t}", tag="HD", bufs=2)
                for k in range(2):
                    nc.tensor.matmul(hd[:], moT[k][:], headW_sb[k][:],
                                     start=(k == 0), stop=False)
                nc.tensor.matmul(hd[:], ones_sb[:], bias_sb[:], start=False, stop=True)
                lg = pp.tile([128, NCLS], F32, name=f"lg{t}", tag=f"lg{t}")
                nc.vector.tensor_copy(lg[:], hd[:, 0:NCLS])
                lg_all.append(lg)
                am = sp.tile([128, 1], F32, name=f"am{t}", tag="am", bufs=2)
                nc.vector.reduce_max(am[:], hd[:, NCLS:NCLS + NH], axis=AX.X)
                nc.sync.dma_start(scr_am[t // 8, ds(128 * (t % 8), 128)], am[:])

            # tail: per-sample softmax over time (sample = partition)
            row = wp.tile([BLOC, L], F32, name="row_am", tag="term", bufs=2)
            nc.sync.dma_start(row[:], scr_am[:, :])
            mx = sp.tile([BLOC, 1], F32, name="mx", tag="mx")
            nc.vector.reduce_max(mx[:], row[:], axis=AX.X)
            nmx = sp.tile([BLOC, 1], F32, name="nmx", tag="nmx")
            nc.vector.tensor_scalar_mul(nmx[:], mx[:], -1.0)
            ex = wp.tile([BLOC, L], F32, name="ex", tag="term", bufs=2)
            esum = sp.tile([BLOC, 1], F32, name="esum", tag="esum")
            nc.scalar.activation(ex[:], row[:], AF.Exp, bias=nmx[:, 0:1],
                                 accum_out=esum[:])
            rec = sp.tile([BLOC, 1], F32, name="rec", tag="rec")
            nc.vector.reciprocal(rec[:], esum[:])
            wx = wp.tile([BLOC, L], F32, name="wx", tag="term", bufs=2)
            nc.vector.scalar_tensor_tensor(wx[:], ex[:], rec[:, 0:1], xmrow[:],
                                           op0=OP.mult, op1=OP.mult)
            for b in range(BLOC):
                nc.sync.dma_start(scr_wx[b:b + 1, :], wx[b:b + 1, :])

            wxc = []
            for t in range(NCH):
                wc = sp.tile([128, 1], F32, name=f"wxc{t}", tag=f"wxc{t}")
                nc.sync.dma_start(wc[:], scr_wx[t // 8, ds(128 * (t % 8), 128)])
                wxc.append(wc)
            for b in range(BLOC):
                ops = psr.tile([NCLS, 1], F32, name=f"ops{b}", tag="HD", bufs=2)
                for i in range(8):
                    t = 8 * b + i
                    nc.tensor.matmul(ops[:], lg_all[t][:], wxc[t][:],
                                     start=(i == 0), stop=(i == 7))
                oc = sp.tile([NCLS, 1], F32, name=f"oc{b}", tag=f"oc{b}")
                nc.vector.tensor_copy(oc[:], ops[:])
                nc.sync.dma_start(out[b, :], oc[:])

    nc.finalize()
    return nc


_NC_CACHE = None


def _get_module():
    global _NC_CACHE
    if _NC_CACHE is None:
        _NC_CACHE = _build_module()
    return _NC_CACHE


def _pos_emb_T():
    pos = np.arange(L, dtype=np.float32)[:, None]
    div = np.exp(np.arange(0, DM, 2, dtype=np.float32) * (-np.log(10000.0) / DM))
    pe = np.zeros((L, DM), np.float32)
    pe[:, 0::2] = np.sin(pos * div)
    pe[:, 1::2] = np.cos(pos * div)
    return pe.T.copy()


def _prep_inputs(inputs):
    import ml_dtypes
    f = lambda x: np.ascontiguousarray(np.asarray(x, dtype=np.float32))
    bf = lambda x: np.ascontiguousarray(x).astype(ml_dtypes.bfloat16)
    tokW = f(inputs["tok_conv_w"])                        # [DM, CIN, EK]
    inW = f(inputs["in_proj_w"])                          # [2DI, DM]
    cvw = f(inputs["dconv_w"])[:, 0, :]                   # [DI, DC]
    # convWT[(j,m), d] = in_proj_w[d, m] * dconv_w[d, j]
    convWT = (inW[:DI][None, :, :] * cvw.T[:, :, None]).transpose(0, 2, 1)  # [DC, DM, DI]
    convWT = np.ascontiguousarray(convWT.reshape(DC * DM, DI))
    attnb = f(inputs["attn_b"])
    brow = np.zeros((1, NCLS + NH), np.float32)
    brow[0, NCLS:] = attnb
    shared = {
        "tok_lhsT": bf(np.transpose(tokW, (2, 1, 0)).reshape(KD, DM)),
        "peT": _pos_emb_T(),
        "inWzT": bf(inW[DI:].T),
        "convWT": bf(convWT),
        "dcb": f(inputs["dconv_b"]).reshape(DI, 1),
        "xprojWT": bf(f(inputs["x_proj_w"]).T[:, list(range(DTR)) + list(range(DTR + DS, DTR + 2 * DS)) + list(range(DTR, DTR + DS))]),
        "dtWT": bf(f(inputs["dt_proj_w"]).T),
        "dtb": f(inputs["dt_proj_b"]).reshape(DI, 1),
        "Amat": (-np.exp(f(inputs["A_log"]))).astype(np.float32),
        "Dv": f(inputs["Dvec"]).reshape(DI, 1),
        "WoutT": f(inputs["out_proj_w"]).T.astype(ml_dtypes.bfloat16).copy(),
        "lng_bc": bf(np.broadcast_to(f(inputs["ln_g"]), (128, DM)).copy()),
        "lnb_bc": np.broadcast_to(f(inputs["ln_b"]), (128, DM)).copy(),
        "headWT": bf(np.concatenate([f(inputs["cls_w"]).T, f(inputs["attn_w"]).T], axis=1)),
        "biasrow": bf(brow),
        "onesrow": bf(np.ones((1, 128), np.float32)),
        "onec": np.ones((128, 1), np.float32),
        "epsc": np.full((128, 1), 1e-5, np.float32),
    }
    xTall = np.ascontiguousarray(f(inputs["x_enc"]).transpose(0, 2, 1))  # [B, CIN, L]
    xm = f(inputs["x_mark_enc"])
    per_core = []
    for c in range(NCORES):
        m = dict(shared)
        m["xT"] = np.ascontiguousarray(xTall[c * BLOC:(c + 1) * BLOC]).astype(ml_dtypes.bfloat16)
        m["xmark2"] = np.ascontiguousarray(xm[c * BLOC:(c + 1) * BLOC]).astype(ml_dtypes.bfloat16)
        per_core.append(m)
    return per_core


def kernel(**inputs) -> np.ndarray:
    from concourse.bass_utils import run_bass_kernel_spmd

    nc = _get_module()
    in_maps = _prep_inputs(inputs)
    res = run_bass_kernel_spmd(nc, in_maps, core_ids=list(range(NCORES)))
    return np.concatenate([res.results[c]["out"] for c in range(NCORES)], axis=0)


# revision 18
# speedup vs baseline: 1.0660x; 1.0660x over previous
"""Trainium2 Bass kernel for nn_Model_14998025797662 (Mamba-TimeVariant classifier).

Self-contained: hardcodes shapes. Data-parallel over batch: 16 samples ->
8 cores x 2 samples. Layout: channels-on-partitions, time-on-free.

v2: both per-core samples are packed along the free axis (T2 = 2048) so every
elementwise/scan op covers both samples in one instruction. The scan resets at
the sample boundary via a poisoned dt column (dt=+30 -> dA=exp(A*30)=0).
Engine budget: SSM scans pinned on DVE; dA exponentials on ACT; dBu/C-term
muls split DVE(bf16 2x)/Pool; the 60 state-accumulate adds run as SWDGE
accumulate-DMAs (free of engine time). B/C broadcasts via DRAM-row bounce.
Head transposes via xbar DMA-transpose. LayerNorm stats batched across the 16
time chunks to kill the serial scalar chain.
"""

import numpy as np

import concourse.bacc as bacc
import concourse.bass as bass
from concourse import mybir
from concourse.bass import ds, ts
from concourse.tile import TileContext

F32 = mybir.dt.float32
BF16 = mybir.dt.bfloat16
AF = mybir.ActivationFunctionType
OP = mybir.AluOpType
AX = mybir.AxisListType

B, L, CIN = 16, 1024, 12
DM, DS, DC, DI, DTR = 256, 16, 4, 512, 16
NCLS, NH, EK = 10, 8, 3
NCORES = 8
BLOC = B // NCORES          # 2 samples per core
T2 = BLOC * L               # 2048 combined free axis
NDT = DI // 128             # 4 d-tiles
NCH = T2 // 128             # 16 time chunks
KD = EK * CIN               # 36
PAD = DC - 1                # 3 pad cols per sample for the causal conv
EW = T2 + BLOC * PAD        # 2054 emb width


def _off(n):
    """emb col offset of 512-chunk n (pads at [0:3] and [1027:1030])."""
    return PAD + n * 512 + PAD * (n >= 2)


def _pool_dbu(s, d):
    # dBu feeds the DVE scan directly; keeping it on DVE avoids scan stalls
    return False


def _pool_term(s, d):
    # C-side terms are off the critical path (they feed SWDGE accum DMAs);
    # keep ~44 on Pool, 20 on DVE (SWDGE desc-gen also eats Pool time)
    if d == (s % 4):
        return False
    if s % 4 == 3 and d == ((s + 1) % 4):
        return False
    return True


def _patch_act_tables():
    """Bias ACT table-set selection so Exp and Ln resolve to the same set
    (avoids per-op table thrash). Idempotent."""
    import concourse.bacc as _bacc
    import concourse.hw_specs as _hw
    if getattr(_bacc, "_ant_act_tables_patched", False):
        return
    _orig = _hw.get_activation_tables

    def patched(arch):
        t = _orig(arch)
        both = None
        for name, fns in t.items():
            sn = {str(x).split(".")[-1] for x in fns}
            if "Exp" in sn and "Ln" in sn:
                both = name
                break
        if both is not None:
            for name, fns in t.items():
                if name == both:
                    continue
                fns.discard(mybir.ActivationFunctionType.Exp)
                fns.discard(mybir.ActivationFunctionType.Ln)
        return t

    _bacc.get_activation_tables = patched
    _bacc._ant_act_tables_patched = True


def _build_module():
    _patch_act_tables()
    nc = bacc.Bacc("TRN2", target_bir_lowering=False)

    def din(name, shape, dt=F32):
        return nc.dram_tensor(name, shape, dt, kind="ExternalInput")

    xT = din("xT", [BLOC, CIN, L], BF16)
    xmark2 = din("xmark2", [BLOC, L], BF16)
    tok_lhsT = din("tok_lhsT", [KD, DM], BF16)
    peT = din("peT", [DM, L])
    inWzT = din("inWzT", [DM, DI], BF16)    # z half of in_proj
    convWT = din("convWT", [DC * DM, DI], BF16)
    dcb = din("dcb", [DI, 1])
    xprojWT = din("xprojWT", [DI, DTR + 2 * DS], BF16)
    dtWT = din("dtWT", [DTR, DI], BF16)
    dtb = din("dtb", [DI, 1])
    Amat = din("Amat", [DI, DS])
    Dv = din("Dv", [DI, 1])
    WoutT = din("WoutT", [DI, DM + 1], BF16)
    lng_bc = din("lng_bc", [128, DM], BF16)
    lnb_bc = din("lnb_bc", [128, DM])
    headWT = din("headWT", [DM, NCLS + NH], BF16)
    biasrow = din("biasrow", [1, NCLS + NH], BF16)
    onesrow = din("onesrow", [1, 128], BF16)
    onec = din("onec", [128, 1])
    epsc = din("epsc", [128, 1])

    out = nc.dram_tensor("out", [BLOC, NCLS], F32, kind="ExternalOutput")
    scr_bc = nc.dram_tensor("scr_bc", [2 * DS, T2], BF16)   # rows 0:16 C, 16:32 B
    scr_am = nc.dram_tensor("scr_am", [BLOC, L], F32)
    scr_wx = nc.dram_tensor("scr_wx", [BLOC, L], F32)

    with TileContext(nc) as tc:
        with (
            tc.tile_pool(name="const", bufs=1) as cp,
            tc.tile_pool(name="persist", bufs=1) as pp,
            tc.tile_pool(name="work", bufs=2) as wp,
            tc.tile_pool(name="small", bufs=2) as sp,
            tc.tile_pool(name="psumr", bufs=3, space="PSUM") as psr,
        ):
            def cload(name, shape, src, dt=F32):
                t = cp.tile(shape, dt, name=f"c_{name}")
                nc.sync.dma_start(t[:], src)
                return t

            # inputs + first-use weights load first so stage A starts early
            tokW_sb = cload("tokW", [KD, DM], tok_lhsT[:], BF16)
            rhs36 = pp.tile([KD, T2], BF16, name="rhs36", tag="skinny", bufs=2)
            for b in range(BLOC):
                c0 = b * L
                nc.sync.dma_start(rhs36[12:24, c0:c0 + L], xT[b, :, :])
                nc.sync.dma_start(rhs36[0:12, c0 + 1:c0 + L], xT[b, :, 0:L - 1])
                nc.sync.dma_start(rhs36[0:12, c0:c0 + 1], xT[b, :, 0:1])
                nc.sync.dma_start(rhs36[24:36, c0:c0 + L - 1], xT[b, :, 1:L])
                nc.sync.dma_start(rhs36[24:36, c0 + L - 1:c0 + L], xT[b, :, L - 1:L])
            pe_sb = []
            for m in range(2):
                # staged in the dbu work-tag ring (dead before stage B uses it)
                pt = wp.tile([128, L], F32, name=f"pe{m}", tag="dbu", bufs=2)
                nc.sync.dma_start(pt[:], peT[ts(m, 128), :])
                pe_sb.append(pt)
            inWz_sb = [cload(f"inWz{k}", [128, DI], inWzT[ts(k, 128), :], BF16) for k in range(2)]
            convW_sb = [cload(f"cvW{k}", [128, DI], convWT[ts(k, 128), :], BF16) for k in range(8)]
            dcb_sb = [cload(f"dcb{d}", [128, 1], dcb[ts(d, 128), :]) for d in range(NDT)]
            xprojW_sb = [cload(f"xpW{d}", [128, DTR + 2 * DS], xprojWT[ts(d, 128), :], BF16) for d in range(NDT)]
            dtW_sb = cload("dtW", [DTR, DI], dtWT[:], BF16)
            dtb_sb = [cload(f"dtb{d}", [128, 1], dtb[ts(d, 128), :]) for d in range(NDT)]
            A_sb = [cload(f"A{d}", [128, DS], Amat[ts(d, 128), :]) for d in range(NDT)]
            Dv_sb = [cload(f"Dv{d}", [128, 1], Dv[ts(d, 128), :]) for d in range(NDT)]
            Wout_sb = [cload(f"Wo{d}", [128, DM + 1], WoutT[ts(d, 128), :], BF16) for d in range(NDT)]
            lng_sb = cload("lng", [128, DM], lng_bc[:], BF16)
            lnb_sb = cload("lnb", [128, DM], lnb_bc[:])
            headW_sb = [cload(f"hW{k}", [128, NCLS + NH], headWT[ts(k, 128), :], BF16) for k in range(2)]
            bias_sb = cload("biasrow", [1, NCLS + NH], biasrow[:], BF16)
            ones_sb = cload("onesrow", [1, 128], onesrow[:], BF16)
            one_sb = cload("onec", [128, 1], onec[:])
            eps_sb = cload("epsc", [128, 1], epsc[:])
            xmrow = wp.tile([BLOC, L], BF16, name="xmrow", tag="xmk", bufs=1)
            nc.sync.dma_start(xmrow[:], xmark2[:, :])

            # ======== stage A ========

            emb_sb = [pp.tile([128, EW], BF16, name=f"emb{m}", tag="embh", bufs=3) for m in range(2)]
            for m in range(2):
                nc.vector.memset(emb_sb[m][:, 0:PAD], 0.0)
                nc.vector.memset(emb_sb[m][:, PAD + L:PAD + L + PAD], 0.0)

            sz_sb = [pp.tile([128, T2], BF16, name=f"sz{d}", tag=f"sz{d}") for d in range(NDT)]
            u0_sb = [pp.tile([128, T2], BF16, name=f"u0{d}", tag=f"u0{d}") for d in range(NDT)]
            xdbl_sb = pp.tile([DTR + 2 * DS, T2], BF16, name="xdbl", tag="skinny", bufs=2)
            dt_sb = [pp.tile([128, T2], BF16, name=f"dt{d}", tag=f"dt{d}") for d in range(NDT)]

            # pass 1 (silu act-table): emb, z-silu, conv-silu per chunk
            for n in range(4):
                o = _off(n)
                # emb chunk: tok conv + positional embedding
                for m in range(2):
                    ps = psr.tile([128, 512], F32, name=f"eps{m}{n}", tag="ps512")
                    nc.tensor.matmul(ps[:], tokW_sb[:, ts(m, 128)], rhs36[:, ts(n, 512)],
                                     start=True, stop=True)
                    nc.vector.tensor_add(emb_sb[m][:, ds(o, 512)], ps[:],
                                         pe_sb[m][:, ds((n % 2) * 512, 512)])
                # z half -> silu
                for d in range(NDT):
                    ps = psr.tile([128, 512], F32, name=f"z{d}{n}", tag="ps512")
                    for k in range(2):
                        nc.tensor.matmul(ps[:], inWz_sb[k][:, ts(d, 128)],
                                         emb_sb[k][:, ds(o, 512)],
                                         start=(k == 0), stop=(k == 1))
                    nc.scalar.activation(sz_sb[d][:, ts(n, 512)], ps[:], AF.Silu)
                # fused causal conv of in_proj x-half -> silu
                for d in range(NDT):
                    ps = psr.tile([128, 512], F32, name=f"u{d}{n}", tag="ps512")
                    for k in range(8):
                        j = k // 2
                        nc.tensor.matmul(ps[:], convW_sb[k][:, ts(d, 128)],
                                         emb_sb[k % 2][:, ds(o - PAD + j, 512)],
                                         start=(k == 0), stop=(k == 7))
                    nc.scalar.activation(u0_sb[d][:, ts(n, 512)], ps[:], AF.Silu,
                                         bias=dcb_sb[d][:, 0:1])
            # pass 2 (no act table): x_proj -> x_dblT
            for n in range(4):
                ps = psr.tile([DTR + 2 * DS, 512], F32, name=f"xd{n}", tag="ps512")
                for k in range(NDT):
                    nc.tensor.matmul(ps[:], xprojW_sb[k][:], u0_sb[k][:, ts(n, 512)],
                                     start=(k == 0), stop=(k == NDT - 1))
                nc.scalar.copy(xdbl_sb[:, ts(n, 512)], ps[:])
            # pass 3 (exp/ln act-table): dt = softplus
            for n in range(4):
                for d in range(NDT):
                    ps = psr.tile([128, 512], F32, name=f"dtp{d}{n}", tag="ps512")
                    nc.tensor.matmul(ps[:], dtW_sb[:, ts(d, 128)], xdbl_sb[0:DTR, ts(n, 512)],
                                     start=True, stop=True)
                    esp = psr.tile([128, 512], F32, name=f"esp{d}{n}", tag="ps512")
                    nc.scalar.activation(esp[:], ps[:], AF.Exp, bias=dtb_sb[d][:, 0:1])
                    nc.scalar.activation(dt_sb[d][:, ts(n, 512)], esp[:], AF.Ln,
                                         bias=one_sb[:, 0:1])

            # stage B/C broadcast rows to DRAM (single wide write)
            nc.sync.dma_start(scr_bc[:, :], xdbl_sb[DTR:DTR + 2 * DS, :])

            # w = dt*u, then poison dt col at the sample boundary so dA goes
            # to 0 there (scan state reset)
            wT_sb = [pp.tile([128, T2], BF16, name=f"w{d}", tag=f"w{d}") for d in range(NDT)]
            for d in range(NDT):
                nc.vector.tensor_mul(wT_sb[d][:], dt_sb[d][:], u0_sb[d][:])
            for d in range(NDT):
                nc.vector.memset(dt_sb[d][:, L:L + 1], 30.0)

            # ======== stage B: 16 SSM states ========
            acc = [pp.tile([128, T2], BF16, name=f"acc{d}", tag=f"acc{d}") for d in range(NDT)]
            for s in range(DS):
                bbc = wp.tile([128, T2], BF16, name=f"bbc{s}", tag="bbc", bufs=2)
                nc.sync.dma_start(bbc[:], scr_bc[DS + s:DS + s + 1, :].to_broadcast((128, T2)))
                cbc = wp.tile([128, T2], BF16, name=f"cbc{s}", tag="cbc", bufs=2)
                nc.sync.dma_start(cbc[:], scr_bc[s:s + 1, :].to_broadcast((128, T2)))
                dAs, dBus, hs = [], [], []
                for d in range(NDT):
                    dA = wp.tile([128, T2], F32, name=f"dA{s}{d}", tag="dA", bufs=3)
                    nc.scalar.activation(dA[:], dt_sb[d][:], AF.Exp, scale=A_sb[d][:, s:s + 1])
                    dAs.append(dA)
                for d in range(NDT):
                    dBu = wp.tile([128, T2], BF16, name=f"dBu{s}{d}", tag="dbu", bufs=2)
                    eng = nc.gpsimd if _pool_dbu(s, d) else nc.vector
                    eng.tensor_mul(dBu[:], wT_sb[d][:], bbc[:])
                    dBus.append(dBu)
                for d in range(NDT):
                    h = wp.tile([128, T2], BF16, name=f"h{s}{d}", tag="embh", bufs=3)
                    nc.vector.tensor_tensor_scan(h[:], dAs[d][:], dBus[d][:], 0.0,
                                                 op0=OP.mult, op1=OP.add)
                    hs.append(h)
                for d in range(NDT):
                    eng = nc.gpsimd if _pool_term(s, d) else nc.vector
                    if s == 0:
                        eng.tensor_mul(acc[d][:], hs[d][:], cbc[:])
                    else:
                        term = wp.tile([128, T2], BF16, name=f"term{s}{d}", tag="term", bufs=2)
                        eng.tensor_mul(term[:], hs[d][:], cbc[:])
                        nc.gpsimd.dma_start(acc[d][:], term[:], accum_op=OP.add)

            # ytot = (acc + u*D) * sz, stored back into the w tiles
            for d in range(NDT):
                t1 = wp.tile([128, T2], BF16, name=f"yt1{d}", tag="dbu", bufs=2)
                nc.vector.scalar_tensor_tensor(t1[:], u0_sb[d][:], Dv_sb[d][:, 0:1],
                                               acc[d][:], op0=OP.mult, op1=OP.add)
                nc.vector.tensor_mul(wT_sb[d][:], t1[:], sz_sb[d][:])

            # ======== stage C ========
            mo_ps, mosb = [], []
            ssum_all = sp.tile([128, NCH], F32, name="ssum_all", tag="ssum", bufs=1)
            sqs_all = sp.tile([128, NCH], F32, name="sqs_all", tag="sqs", bufs=1)
            for t in range(NCH):
                mp = psr.tile([128, DM + 1], F32, name=f"mo{t}", tag="MO", bufs=3)
                for d in range(NDT):
                    nc.tensor.matmul(mp[:], wT_sb[d][:, ts(t, 128)], Wout_sb[d][:],
                                     start=(d == 0), stop=(d == NDT - 1))
                ms = pp.tile([128, DM], BF16, name=f"mosb{t}", tag=f"mos{t}")
                nc.vector.tensor_copy(ms[:], mp[:, 0:DM])
                nc.vector.tensor_copy(ssum_all[:, t:t + 1], mp[:, DM:DM + 1])
                sq = wp.tile([128, DM], BF16, name=f"sq{t}", tag="sqo", bufs=2)
                nc.vector.scalar_tensor_tensor(sq[:], ms[:], 1.0, ms[:],
                                               op0=OP.mult, op1=OP.mult,
                                               accum_out=sqs_all[:, t:t + 1])
                mosb.append(ms)

            # batched LN stats over all 16 chunks
            mun = sp.tile([128, NCH], F32, name="mun", tag="mun")
            nc.vector.tensor_scalar_mul(mun[:], ssum_all[:], -1.0 / DM)
            m2t = sp.tile([128, NCH], F32, name="m2t", tag="m2t")
            nc.vector.tensor_scalar_mul(m2t[:], sqs_all[:], 1.0 / DM)
            msq = sp.tile([128, NCH], F32, name="msq", tag="msq")
            nc.vector.tensor_mul(msq[:], mun[:], mun[:])
            var = sp.tile([128, NCH], F32, name="var", tag="var")
            nc.vector.tensor_sub(var[:], m2t[:], msq[:])
            lnv = sp.tile([128, NCH], F32, name="lnv", tag="lnv")
            nc.scalar.activation(lnv[:], var[:], AF.Ln, bias=eps_sb[:, 0:1])
            rstd = sp.tile([128, NCH], F32, name="rstd", tag="rstd")
            nc.scalar.activation(rstd[:], lnv[:], AF.Exp, scale=-0.5)
            nmr = sp.tile([128, NCH], F32, name="nmr", tag="nmr")
            nc.vector.tensor_mul(nmr[:], mun[:], rstd[:])

            lg_all = []
            for t in range(NCH):
                xn = wp.tile([128, DM], F32, name=f"xn{t}", tag="xn", bufs=2)
                nc.scalar.activation(xn[:], mosb[t][:], AF.Identity,
                                     bias=nmr[:, t:t + 1], scale=rstd[:, t:t + 1])
                t1 = wp.tile([128, DM], F32, name=f"t1{t}", tag="t1", bufs=2)
                nc.vector.tensor_mul(t1[:], xn[:], lng_sb[:])
                t2 = wp.tile([128, DM], F32, name=f"t2{t}", tag="t2", bufs=2)
                nc.gpsimd.tensor_add(t2[:], t1[:], lnb_sb[:])
                mam = wp.tile([128, DM], BF16, name=f"mam{t}", tag="mam", bufs=3)
                nc.scalar.activation(mam[:], t2[:], AF.Silu)
                moT = [wp.tile([128, 128], BF16, name=f"moT{t}{m}", tag="moT", bufs=3)
                       for m in range(2)]
                for m in range(2):
                    nc.scalar.dma_start(moT[m][:], mam[:, ts(m, 128)], transpose=True)
                hd = psr.tile([128, NCLS + NH], F32, name=f"hd{t}", tag="HD", bufs=2)
                for k in range(2):
                    nc.tensor.matmul(hd[:], moT[k][:], headW_sb[k][:],
                                     start=(k == 0), stop=False)
                nc.tensor.matmul(hd[:], ones_sb[:], bias_sb[:], start=False, stop=True)
                lg = pp.tile([128, NCLS], F32, name=f"lg{t}", tag=f"lg{t}")
                nc.vector.tensor_copy(lg[:], hd[:, 0:NCLS])
                lg_all.append(lg)
                am = sp.tile([128, 1], F32, name=f"am{t}", tag="am", bufs=2)
                nc.vector.reduce_max(am[:], hd[:, NCLS:NCLS + NH], axis=AX.X)
                nc.scalar.dma_start(scr_am[t // 8, ds(128 * (t % 8), 128)], am[:])

            # tail: per-sample softmax over time (sample = partition)
            row = wp.tile([BLOC, L], F32, name="row_am", tag="term", bufs=2)
            nc.sync.dma_start(row[:], scr_am[:, :])
            mx = sp.tile([BLOC, 1], F32, name="mx", tag="mx")
            nc.vector.reduce_max(mx[:], row[:], axis=AX.X)
            nmx = sp.tile([BLOC, 1], F32, name="nmx", tag="nmx")
            nc.vector.tensor_scalar_mul(nmx[:], mx[:], -1.0)
            ex = wp.tile([BLOC, L], F32, name="ex", tag="term", bufs=2)
            esum = sp.tile([BLOC, 1], F32, name="esum", tag="esum")
            nc.scalar.activation(ex[:], row[:], AF.Exp, bias=nmx[:, 0:1],
                                 accum_out=esum[:])
            rec = sp.tile([BLOC, 1], F32, name="rec", tag="rec")
            nc.vector.reciprocal(rec[:], esum[:])
            wx = wp.tile([BLOC, L], F32, name="wx", tag="term", bufs=2)
            nc.vector.scalar_tensor_tensor(wx[:], ex[:], rec[:, 0:1], xmrow[:],
                                           op0=OP.mult, op1=OP.mult)
            for b in range(BLOC):
                nc.sync.dma_start(scr_wx[b:b + 1, :], wx[b:b + 1, :])

            wxc = []
            for t in range(NCH):
                wc = sp.tile([128, 1], F32, name=f"wxc{t}", tag=f"wxc{t}")
                nc.scalar.dma_start(wc[:], scr_wx[t // 8, ds(128 * (t % 8), 128)])
                wxc.append(wc)
            for b in range(BLOC):
                ops = psr.tile([NCLS, 1], F32, name=f"ops{b}", tag="HD", bufs=2)
                for i in range(8):
                    t = 8 * b + i
                    nc.tensor.matmul(ops[:], lg_all[t][:], wxc[t][:],
                                     start=(i == 0), stop=(i == 7))
                oc = sp.tile([NCLS, 1], F32, name=f"oc{b}", tag=f"oc{b}")
                nc.vector.tensor_copy(oc[:], ops[:])
                nc.sync.dma_start(out[b, :], oc[:])

    nc.finalize()
    return nc


_NC_CACHE = None


def _get_module():
    global _NC_CACHE
    if _NC_CACHE is None:
        _NC_CACHE = _build_module()
    return _NC_CACHE


def _pos_emb_T():
    pos = np.arange(L, dtype=np.float32)[:, None]
    div = np.exp(np.arange(0, DM, 2, dtype=np.float32) * (-np.log(10000.0) / DM))
    pe = np.zeros((L, DM), np.float32)
    pe[:, 0::2] = np.sin(pos * div)
    pe[:, 1::2] = np.cos(pos * div)
    return pe.T.copy()


def _prep_inputs(inputs):
    import ml_dtypes
    f = lambda x: np.ascontiguousarray(np.asarray(x, dtype=np.float32))
    bf = lambda x: np.ascontiguousarray(x).astype(ml_dtypes.bfloat16)
    tokW = f(inputs["tok_conv_w"])                        # [DM, CIN, EK]
    inW = f(inputs["in_proj_w"])                          # [2DI, DM]
    cvw = f(inputs["dconv_w"])[:, 0, :]                   # [DI, DC]
    # convWT[(j,m), d] = in_proj_w[d, m] * dconv_w[d, j]
    convWT = (inW[:DI][None, :, :] * cvw.T[:, :, None]).transpose(0, 2, 1)  # [DC, DM, DI]
    convWT = np.ascontiguousarray(convWT.reshape(DC * DM, DI))
    attnb = f(inputs["attn_b"])
    brow = np.zeros((1, NCLS + NH), np.float32)
    brow[0, NCLS:] = attnb
    shared = {
        "tok_lhsT": bf(np.transpose(tokW, (2, 1, 0)).reshape(KD, DM)),
        "peT": _pos_emb_T(),
        "inWzT": bf(inW[DI:].T),
        "convWT": bf(convWT),
        "dcb": f(inputs["dconv_b"]).reshape(DI, 1),
        "xprojWT": bf(f(inputs["x_proj_w"]).T[:, list(range(DTR)) + list(range(DTR + DS, DTR + 2 * DS)) + list(range(DTR, DTR + DS))]),
        "dtWT": bf(f(inputs["dt_proj_w"]).T),
        "dtb": f(inputs["dt_proj_b"]).reshape(DI, 1),
        "Amat": (-np.exp(f(inputs["A_log"]))).astype(np.float32),
        "Dv": f(inputs["Dvec"]).reshape(DI, 1),
        "WoutT": bf(np.concatenate([f(inputs["out_proj_w"]).T,
                                    f(inputs["out_proj_w"]).T.sum(axis=1, keepdims=True)], axis=1)),
        "lng_bc": bf(np.broadcast_to(f(inputs["ln_g"]), (128, DM)).copy()),
        "lnb_bc": np.broadcast_to(f(inputs["ln_b"]), (128, DM)).copy(),
        "headWT": bf(np.concatenate([f(inputs["cls_w"]).T, f(inputs["attn_w"]).T], axis=1)),
        "biasrow": bf(brow),
        "onesrow": bf(np.ones((1, 128), np.float32)),
        "onec": np.ones((128, 1), np.float32),
        "epsc": np.full((128, 1), 1e-5, np.float32),
    }
    xTall = np.ascontiguousarray(f(inputs["x_enc"]).transpose(0, 2, 1))  # [B, CIN, L]
    xm = f(inputs["x_mark_enc"])
    per_core = []
    for c in range(NCORES):
        m = dict(shared)
        m["xT"] = np.ascontiguousarray(xTall[c * BLOC:(c + 1) * BLOC]).astype(ml_dtypes.bfloat16)
        m["xmark2"] = np.ascontiguousarray(xm[c * BLOC:(c + 1) * BLOC]).astype(ml_dtypes.bfloat16)
        per_core.append(m)
    return per_core


def kernel(**inputs) -> np.ndarray:
    from concourse.bass_utils import run_bass_kernel_spmd

    nc = _get_module()
    in_maps = _prep_inputs(inputs)
    res = run_bass_kernel_spmd(nc, in_maps, core_ids=list(range(NCORES)))
    return np.concatenate([res.results[c]["out"] for c in range(NCORES)], axis=0)


# revision 19
# speedup vs baseline: 1.1096x; 1.0409x over previous
"""Trainium2 Bass kernel for nn_Model_14998025797662 (Mamba-TimeVariant classifier).

Self-contained: hardcodes shapes. Data-parallel over batch: 16 samples ->
8 cores x 2 samples. Layout: channels-on-partitions, time-on-free.

v2: both per-core samples are packed along the free axis (T2 = 2048) so every
elementwise/scan op covers both samples in one instruction. The scan resets at
the sample boundary via a poisoned dt column (dt=+30 -> dA=exp(A*30)=0).
Engine budget: SSM scans pinned on DVE; dA exponentials on ACT; dBu/C-term
muls split DVE(bf16 2x)/Pool; the 60 state-accumulate adds run as SWDGE
accumulate-DMAs (free of engine time). B/C broadcasts via DRAM-row bounce.
Head transposes via xbar DMA-transpose. LayerNorm stats batched across the 16
time chunks to kill the serial scalar chain.
"""

import numpy as np

import concourse.bacc as bacc
import concourse.bass as bass
from concourse import mybir
from concourse.bass import ds, ts
from concourse.tile import TileContext

F32 = mybir.dt.float32
BF16 = mybir.dt.bfloat16
AF = mybir.ActivationFunctionType
OP = mybir.AluOpType
AX = mybir.AxisListType

B, L, CIN = 16, 1024, 12
DM, DS, DC, DI, DTR = 256, 16, 4, 512, 16
NCLS, NH, EK = 10, 8, 3
NCORES = 8
BLOC = B // NCORES          # 2 samples per core
T2 = BLOC * L               # 2048 combined free axis
NDT = DI // 128             # 4 d-tiles
NCH = T2 // 128             # 16 time chunks
KD = EK * CIN               # 36
PAD = DC - 1                # 3 pad cols per sample for the causal conv
EW = T2 + BLOC * PAD        # 2054 emb width


def _off(n):
    """emb col offset of 512-chunk n (pads at [0:3] and [1027:1030])."""
    return PAD + n * 512 + PAD * (n >= 2)


def _pool_dbu(s, d):
    # dBu feeds the DVE scan directly; keeping it on DVE avoids scan stalls
    return False


def _pool_term(s, d):
    # C-side terms are off the critical path (they feed SWDGE accum DMAs);
    # keep ~44 on Pool, 20 on DVE (SWDGE desc-gen also eats Pool time)
    if d == (s % 4):
        return False
    if s % 4 == 3 and d == ((s + 1) % 4):
        return False
    return True


def _patch_act_tables():
    """Bias ACT table-set selection so Exp and Ln resolve to the same set
    (avoids per-op table thrash). Idempotent."""
    import concourse.bacc as _bacc
    import concourse.hw_specs as _hw
    if getattr(_bacc, "_ant_act_tables_patched", False):
        return
    _orig = _hw.get_activation_tables

    def patched(arch):
        t = _orig(arch)
        both = None
        for name, fns in t.items():
            sn = {str(x).split(".")[-1] for x in fns}
            if "Exp" in sn and "Ln" in sn:
                both = name
                break
        if both is not None:
            for name, fns in t.items():
                if name == both:
                    continue
                fns.discard(mybir.ActivationFunctionType.Exp)
                fns.discard(mybir.ActivationFunctionType.Ln)
        return t

    _bacc.get_activation_tables = patched
    _bacc._ant_act_tables_patched = True


def _build_module():
    _patch_act_tables()
    nc = bacc.Bacc("TRN2", target_bir_lowering=False)

    def din(name, shape, dt=F32):
        return nc.dram_tensor(name, shape, dt, kind="ExternalInput")

    xT = din("xT", [BLOC, CIN, L], BF16)
    xmark2 = din("xmark2", [BLOC, L], BF16)
    tok_lhsT = din("tok_lhsT", [KD, DM], BF16)
    peT = din("peT", [DM, L])
    inWzT = din("inWzT", [DM, DI], BF16)    # z half of in_proj
    convWT = din("convWT", [DC * DM, DI], BF16)
    dcb = din("dcb", [DI, 1])
    xprojWT = din("xprojWT", [DI, DTR + 2 * DS], BF16)
    dtWT = din("dtWT", [DTR, DI], BF16)
    dtb = din("dtb", [DI, 1])
    Amat = din("Amat", [DI, DS])
    Dv = din("Dv", [DI, 1])
    WoutT = din("WoutT", [DI, DM + 1], BF16)
    lng_bc = din("lng_bc", [128, DM], BF16)
    lnb_bc = din("lnb_bc", [128, DM])
    headWT = din("headWT", [DM, NCLS + NH], BF16)
    biasrow = din("biasrow", [1, NCLS + NH], BF16)
    onesrow = din("onesrow", [1, 128], BF16)
    onec = din("onec", [128, 1])
    epsc = din("epsc", [128, 1])

    out = nc.dram_tensor("out", [BLOC, NCLS], F32, kind="ExternalOutput")
    scr_bc = nc.dram_tensor("scr_bc", [2 * DS, T2], BF16)   # rows 0:16 C, 16:32 B
    scr_am = nc.dram_tensor("scr_am", [BLOC, L], F32)
    scr_wx = nc.dram_tensor("scr_wx", [BLOC, L], F32)

    with TileContext(nc) as tc:
        with (
            tc.tile_pool(name="const", bufs=1) as cp,
            tc.tile_pool(name="persist", bufs=1) as pp,
            tc.tile_pool(name="work", bufs=2) as wp,
            tc.tile_pool(name="small", bufs=2) as sp,
            tc.tile_pool(name="psumr", bufs=3, space="PSUM") as psr,
        ):
            def cload(name, shape, src, dt=F32):
                t = cp.tile(shape, dt, name=f"c_{name}")
                nc.sync.dma_start(t[:], src)
                return t

            # inputs + first-use weights load first so stage A starts early
            tokW_sb = cload("tokW", [KD, DM], tok_lhsT[:], BF16)
            rhs36 = pp.tile([KD, T2], BF16, name="rhs36", tag="skinny", bufs=2)
            for b in range(BLOC):
                c0 = b * L
                nc.scalar.dma_start(rhs36[12:24, c0:c0 + L], xT[b, :, :])
                nc.scalar.dma_start(rhs36[0:12, c0 + 1:c0 + L], xT[b, :, 0:L - 1])
                nc.scalar.dma_start(rhs36[0:12, c0:c0 + 1], xT[b, :, 0:1])
                nc.scalar.dma_start(rhs36[24:36, c0:c0 + L - 1], xT[b, :, 1:L])
                nc.scalar.dma_start(rhs36[24:36, c0 + L - 1:c0 + L], xT[b, :, L - 1:L])
            pe_sb = []
            for m in range(2):
                # staged in the dbu work-tag ring (dead before stage B uses it)
                pt = wp.tile([128, L], F32, name=f"pe{m}", tag="dbu", bufs=2)
                nc.scalar.dma_start(pt[:], peT[ts(m, 128), :])
                pe_sb.append(pt)
            inWz_sb = [cload(f"inWz{k}", [128, DI], inWzT[ts(k, 128), :], BF16) for k in range(2)]
            convW_sb = [cload(f"cvW{k}", [128, DI], convWT[ts(k, 128), :], BF16) for k in range(8)]
            dcb_sb = [cload(f"dcb{d}", [128, 1], dcb[ts(d, 128), :]) for d in range(NDT)]
            xprojW_sb = [cload(f"xpW{d}", [128, DTR + 2 * DS], xprojWT[ts(d, 128), :], BF16) for d in range(NDT)]
            dtW_sb = cload("dtW", [DTR, DI], dtWT[:], BF16)
            dtb_sb = [cload(f"dtb{d}", [128, 1], dtb[ts(d, 128), :]) for d in range(NDT)]
            A_sb = [cload(f"A{d}", [128, DS], Amat[ts(d, 128), :]) for d in range(NDT)]
            Dv_sb = [cload(f"Dv{d}", [128, 1], Dv[ts(d, 128), :]) for d in range(NDT)]
            Wout_sb = [cload(f"Wo{d}", [128, DM + 1], WoutT[ts(d, 128), :], BF16) for d in range(NDT)]
            lng_sb = cload("lng", [128, DM], lng_bc[:], BF16)
            lnb_sb = cload("lnb", [128, DM], lnb_bc[:])
            headW_sb = [cload(f"hW{k}", [128, NCLS + NH], headWT[ts(k, 128), :], BF16) for k in range(2)]
            bias_sb = cload("biasrow", [1, NCLS + NH], biasrow[:], BF16)
            ones_sb = cload("onesrow", [1, 128], onesrow[:], BF16)
            one_sb = cload("onec", [128, 1], onec[:])
            eps_sb = cload("epsc", [128, 1], epsc[:])
            xmrow = wp.tile([BLOC, L], BF16, name="xmrow", tag="xmk", bufs=1)
            nc.sync.dma_start(xmrow[:], xmark2[:, :])

            # ======== stage A ========

            emb_sb = [pp.tile([128, EW], BF16, name=f"emb{m}", tag="embt", bufs=2) for m in range(2)]
            for m in range(2):
                nc.vector.memset(emb_sb[m][:, 0:PAD], 0.0)
                nc.vector.memset(emb_sb[m][:, PAD + L:PAD + L + PAD], 0.0)

            sz_sb = [pp.tile([128, T2], BF16, name=f"sz{d}", tag=f"sz{d}") for d in range(NDT)]
            u0_sb = [pp.tile([128, T2], BF16, name=f"u0{d}", tag=f"u0{d}") for d in range(NDT)]
            xdbl_sb = pp.tile([DTR + 2 * DS, T2], BF16, name="xdbl", tag="skinny", bufs=2)
            dt_sb = [pp.tile([128, T2], BF16, name=f"dt{d}", tag=f"dt{d}") for d in range(NDT)]

            # pass 1 (silu act-table): emb, z-silu, conv-silu per chunk
            for n in range(4):
                o = _off(n)
                # emb chunk: tok conv + positional embedding
                for m in range(2):
                    ps = psr.tile([128, 512], F32, name=f"eps{m}{n}", tag="ps512")
                    nc.tensor.matmul(ps[:], tokW_sb[:, ts(m, 128)], rhs36[:, ts(n, 512)],
                                     start=True, stop=True)
                    nc.vector.tensor_add(emb_sb[m][:, ds(o, 512)], ps[:],
                                         pe_sb[m][:, ds((n % 2) * 512, 512)])
                # z half -> silu
                for d in range(NDT):
                    ps = psr.tile([128, 512], F32, name=f"z{d}{n}", tag="ps512")
                    for k in range(2):
                        nc.tensor.matmul(ps[:], inWz_sb[k][:, ts(d, 128)],
                                         emb_sb[k][:, ds(o, 512)],
                                         start=(k == 0), stop=(k == 1))
                    nc.scalar.activation(sz_sb[d][:, ts(n, 512)], ps[:], AF.Silu)
                # fused causal conv of in_proj x-half -> silu
                for d in range(NDT):
                    ps = psr.tile([128, 512], F32, name=f"u{d}{n}", tag="ps512")
                    for k in range(8):
                        j = k // 2
                        nc.tensor.matmul(ps[:], convW_sb[k][:, ts(d, 128)],
                                         emb_sb[k % 2][:, ds(o - PAD + j, 512)],
                                         start=(k == 0), stop=(k == 7))
                    nc.scalar.activation(u0_sb[d][:, ts(n, 512)], ps[:], AF.Silu,
                                         bias=dcb_sb[d][:, 0:1])
            # pass 2 (no act table): x_proj -> x_dblT
            for n in range(4):
                ps = psr.tile([DTR + 2 * DS, 512], F32, name=f"xd{n}", tag="ps512")
                for k in range(NDT):
                    nc.tensor.matmul(ps[:], xprojW_sb[k][:], u0_sb[k][:, ts(n, 512)],
                                     start=(k == 0), stop=(k == NDT - 1))
                nc.scalar.copy(xdbl_sb[:, ts(n, 512)], ps[:])
            # pass 3 (exp/ln act-table): dt = softplus
            for n in range(4):
                for d in range(NDT):
                    ps = psr.tile([128, 512], F32, name=f"dtp{d}{n}", tag="ps512")
                    nc.tensor.matmul(ps[:], dtW_sb[:, ts(d, 128)], xdbl_sb[0:DTR, ts(n, 512)],
                                     start=True, stop=True)
                    esp = psr.tile([128, 512], F32, name=f"esp{d}{n}", tag="ps512")
                    nc.scalar.activation(esp[:], ps[:], AF.Exp, bias=dtb_sb[d][:, 0:1])
                    nc.scalar.activation(dt_sb[d][:, ts(n, 512)], esp[:], AF.Ln,
                                         bias=one_sb[:, 0:1])

            # stage B/C broadcast rows to DRAM (single wide write)
            nc.sync.dma_start(scr_bc[:, :], xdbl_sb[DTR:DTR + 2 * DS, :])

            # w = dt*u, then poison dt col at the sample boundary so dA goes
            # to 0 there (scan state reset)
            wT_sb = [pp.tile([128, T2], BF16, name=f"w{d}", tag=f"w{d}") for d in range(NDT)]
            for d in range(NDT):
                nc.vector.tensor_mul(wT_sb[d][:], dt_sb[d][:], u0_sb[d][:])
            for d in range(NDT):
                nc.vector.memset(dt_sb[d][:, L:L + 1], 30.0)

            # ======== stage B: 16 SSM states ========
            acc = [pp.tile([128, T2], BF16, name=f"acc{d}", tag=f"acc{d}") for d in range(NDT)]
            for s in range(DS):
                bbc = wp.tile([128, T2], BF16, name=f"bbc{s}", tag="bbc", bufs=2)
                nc.sync.dma_start(bbc[:], scr_bc[DS + s:DS + s + 1, :].to_broadcast((128, T2)))
                cbc = wp.tile([128, T2], BF16, name=f"cbc{s}", tag="cbc", bufs=2)
                nc.sync.dma_start(cbc[:], scr_bc[s:s + 1, :].to_broadcast((128, T2)))
                dAs, dBus, hs = [], [], []
                for d in range(NDT):
                    dA = wp.tile([128, T2], F32, name=f"dA{s}{d}", tag="dA", bufs=2)
                    nc.scalar.activation(dA[:], dt_sb[d][:], AF.Exp, scale=A_sb[d][:, s:s + 1])
                    dAs.append(dA)
                for d in range(NDT):
                    dBu = wp.tile([128, T2], BF16, name=f"dBu{s}{d}", tag="dbu", bufs=2)
                    eng = nc.gpsimd if _pool_dbu(s, d) else nc.vector
                    eng.tensor_mul(dBu[:], wT_sb[d][:], bbc[:])
                    dBus.append(dBu)
                for d in range(NDT):
                    h = wp.tile([128, T2], BF16, name=f"h{s}{d}", tag="hh", bufs=4)
                    nc.vector.tensor_tensor_scan(h[:], dAs[d][:], dBus[d][:], 0.0,
                                                 op0=OP.mult, op1=OP.add)
                    hs.append(h)
                for d in range(NDT):
                    eng = nc.gpsimd if _pool_term(s, d) else nc.vector
                    if s == 0:
                        eng.tensor_mul(acc[d][:], hs[d][:], cbc[:])
                    else:
                        term = wp.tile([128, T2], BF16, name=f"term{s}{d}", tag="term", bufs=2)
                        eng.tensor_mul(term[:], hs[d][:], cbc[:])
                        nc.gpsimd.dma_start(acc[d][:], term[:], accum_op=OP.add)

            # ytot = (acc + u*D) * sz, stored back into the w tiles
            for d in range(NDT):
                t1 = wp.tile([128, T2], BF16, name=f"yt1{d}", tag="dbu", bufs=2)
                nc.vector.scalar_tensor_tensor(t1[:], u0_sb[d][:], Dv_sb[d][:, 0:1],
                                               acc[d][:], op0=OP.mult, op1=OP.add)
                nc.vector.tensor_mul(wT_sb[d][:], t1[:], sz_sb[d][:])

            # ======== stage C ========
            mo_ps, mosb = [], []
            ssum_all = sp.tile([128, NCH], F32, name="ssum_all", tag="ssum", bufs=1)
            sqs_all = sp.tile([128, NCH], F32, name="sqs_all", tag="sqs", bufs=1)
            for t in range(NCH):
                mp = psr.tile([128, DM + 1], F32, name=f"mo{t}", tag="MO", bufs=3)
                for d in range(NDT):
                    nc.tensor.matmul(mp[:], wT_sb[d][:, ts(t, 128)], Wout_sb[d][:],
                                     start=(d == 0), stop=(d == NDT - 1))
                ms = pp.tile([128, DM], BF16, name=f"mosb{t}", tag=f"mos{t}")
                nc.scalar.copy(ms[:], mp[:, 0:DM])
                nc.vector.tensor_copy(ssum_all[:, t:t + 1], mp[:, DM:DM + 1])
                sq = wp.tile([128, DM], BF16, name=f"sq{t}", tag="sqo", bufs=2)
                nc.vector.scalar_tensor_tensor(sq[:], ms[:], 1.0, ms[:],
                                               op0=OP.mult, op1=OP.mult,
                                               accum_out=sqs_all[:, t:t + 1])
                mosb.append(ms)

            # batched LN stats over all 16 chunks
            mun = sp.tile([128, NCH], F32, name="mun", tag="mun")
            nc.vector.tensor_scalar_mul(mun[:], ssum_all[:], -1.0 / DM)
            m2t = sp.tile([128, NCH], F32, name="m2t", tag="m2t")
            nc.vector.tensor_scalar_mul(m2t[:], sqs_all[:], 1.0 / DM)
            msq = sp.tile([128, NCH], F32, name="msq", tag="msq")
            nc.vector.tensor_mul(msq[:], mun[:], mun[:])
            var = sp.tile([128, NCH], F32, name="var", tag="var")
            nc.vector.tensor_sub(var[:], m2t[:], msq[:])
            lnv = sp.tile([128, NCH], F32, name="lnv", tag="lnv")
            nc.scalar.activation(lnv[:], var[:], AF.Ln, bias=eps_sb[:, 0:1])
            rstd = sp.tile([128, NCH], F32, name="rstd", tag="rstd")
            nc.scalar.activation(rstd[:], lnv[:], AF.Exp, scale=-0.5)
            nmr = sp.tile([128, NCH], F32, name="nmr", tag="nmr")
            nc.vector.tensor_mul(nmr[:], mun[:], rstd[:])

            lg_all = []
            for t in range(NCH):
                xn = wp.tile([128, DM], BF16, name=f"xn{t}", tag="xn", bufs=2)
                nc.scalar.activation(xn[:], mosb[t][:], AF.Identity,
                                     bias=nmr[:, t:t + 1], scale=rstd[:, t:t + 1])
                t1 = wp.tile([128, DM], BF16, name=f"t1{t}", tag="t1", bufs=2)
                nc.vector.tensor_mul(t1[:], xn[:], lng_sb[:])
                t2 = wp.tile([128, DM], BF16, name=f"t2{t}", tag="t2", bufs=2)
                nc.gpsimd.tensor_add(t2[:], t1[:], lnb_sb[:])
                mam = wp.tile([128, DM], BF16, name=f"mam{t}", tag="mam", bufs=3)
                nc.scalar.activation(mam[:], t2[:], AF.Silu)
                moT = [wp.tile([128, 128], BF16, name=f"moT{t}{m}", tag="moT", bufs=4)
                       for m in range(2)]
                nc.sync.dma_start(moT[0][:], mam[:, 0:128], transpose=True)
                nc.scalar.dma_start(moT[1][:], mam[:, 128:256], transpose=True)
                hd = psr.tile([128, NCLS + NH], F32, name=f"hd{t}", tag="HD", bufs=2)
                for k in range(2):
                    nc.tensor.matmul(hd[:], moT[k][:], headW_sb[k][:],
                                     start=(k == 0), stop=False)
                nc.tensor.matmul(hd[:], ones_sb[:], bias_sb[:], start=False, stop=True)
                lg = pp.tile([128, NCLS], F32, name=f"lg{t}", tag=f"lg{t}")
                nc.vector.tensor_copy(lg[:], hd[:, 0:NCLS])
                lg_all.append(lg)
                am = sp.tile([128, 1], F32, name=f"am{t}", tag="am", bufs=2)
                nc.vector.reduce_max(am[:], hd[:, NCLS:NCLS + NH], axis=AX.X)
                nc.sync.dma_start(scr_am[t // 8, ds(128 * (t % 8), 128)], am[:])

            # tail: per-sample softmax over time (sample = partition)
            row = wp.tile([BLOC, L], F32, name="row_am", tag="term", bufs=2)
            nc.sync.dma_start(row[:], scr_am[:, :])
            mx = sp.tile([BLOC, 1], F32, name="mx", tag="mx")
            nc.vector.reduce_max(mx[:], row[:], axis=AX.X)
            nmx = sp.tile([BLOC, 1], F32, name="nmx", tag="nmx")
            nc.vector.tensor_scalar_mul(nmx[:], mx[:], -1.0)
            ex = wp.tile([BLOC, L], F32, name="ex", tag="term", bufs=2)
            esum = sp.tile([BLOC, 1], F32, name="esum", tag="esum")
            nc.scalar.activation(ex[:], row[:], AF.Exp, bias=nmx[:, 0:1],
                                 accum_out=esum[:])
            rec = sp.tile([BLOC, 1], F32, name="rec", tag="rec")
            nc.vector.reciprocal(rec[:], esum[:])
            wx = wp.tile([BLOC, L], F32, name="wx", tag="term", bufs=2)
            nc.vector.scalar_tensor_tensor(wx[:], ex[:], rec[:, 0:1], xmrow[:],
                                           op0=OP.mult, op1=OP.mult)
            for b in range(BLOC):
                nc.sync.dma_start(scr_wx[b:b + 1, :], wx[b:b + 1, :])

            wxc = []
            for t in range(NCH):
                wc = sp.tile([128, 1], F32, name=f"wxc{t}", tag=f"wxc{t}")
                nc.sync.dma_start(wc[:], scr_wx[t // 8, ds(128 * (t % 8), 128)])
                wxc.append(wc)
            for b in range(BLOC):
                ops = psr.tile([NCLS, 1], F32, name=f"ops{b}", tag="HD", bufs=2)
                for i in range(8):
                    t = 8 * b + i
                    nc.tensor.matmul(ops[:], lg_all[t][:], wxc[t][:],
                                     start=(i == 0), stop=(i == 7))
                oc = sp.tile([NCLS, 1], F32, name=f"oc{b}", tag=f"oc{b}")
                nc.vector.tensor_copy(oc[:], ops[:])
                nc.sync.dma_start(out[b, :], oc[:])

    nc.finalize()
    return nc


_NC_CACHE = None


def _get_module():
    global _NC_CACHE
    if _NC_CACHE is None:
        _NC_CACHE = _build_module()
    return _NC_CACHE


def _pos_emb_T():
    pos = np.arange(L, dtype=np.float32)[:, None]
    div = np.exp(np.arange(0, DM, 2, dtype=np.float32) * (-np.log(10000.0) / DM))
    pe = np.zeros((L, DM), np.float32)
    pe[:, 0::2] = np.sin(pos * div)
    pe[:, 1::2] = np.cos(pos * div)
    return pe.T.copy()


def _prep_inputs(inputs):
    import ml_dtypes
    f = lambda x: np.ascontiguousarray(np.asarray(x, dtype=np.float32))
    bf = lambda x: np.ascontiguousarray(x).astype(ml_dtypes.bfloat16)
    tokW = f(inputs["tok_conv_w"])                        # [DM, CIN, EK]
    inW = f(inputs["in_proj_w"])                          # [2DI, DM]
    cvw = f(inputs["dconv_w"])[:, 0, :]                   # [DI, DC]
    # convWT[(j,m), d] = in_proj_w[d, m] * dconv_w[d, j]
    convWT = (inW[:DI][None, :, :] * cvw.T[:, :, None]).transpose(0, 2, 1)  # [DC, DM, DI]
    convWT = np.ascontiguousarray(convWT.reshape(DC * DM, DI))
    attnb = f(inputs["attn_b"])
    brow = np.zeros((1, NCLS + NH), np.float32)
    brow[0, NCLS:] = attnb
    shared = {
        "tok_lhsT": bf(np.transpose(tokW, (2, 1, 0)).reshape(KD, DM)),
        "peT": _pos_emb_T(),
        "inWzT": bf(inW[DI:].T),
        "convWT": bf(convWT),
        "dcb": f(inputs["dconv_b"]).reshape(DI, 1),
        "xprojWT": bf(f(inputs["x_proj_w"]).T[:, list(range(DTR)) + list(range(DTR + DS, DTR + 2 * DS)) + list(range(DTR, DTR + DS))]),
        "dtWT": bf(f(inputs["dt_proj_w"]).T),
        "dtb": f(inputs["dt_proj_b"]).reshape(DI, 1),
        "Amat": (-np.exp(f(inputs["A_log"]))).astype(np.float32),
        "Dv": f(inputs["Dvec"]).reshape(DI, 1),
        "WoutT": bf(np.concatenate([f(inputs["out_proj_w"]).T,
                                    f(inputs["out_proj_w"]).T.sum(axis=1, keepdims=True)], axis=1)),
        "lng_bc": bf(np.broadcast_to(f(inputs["ln_g"]), (128, DM)).copy()),
        "lnb_bc": np.broadcast_to(f(inputs["ln_b"]), (128, DM)).copy(),
        "headWT": bf(np.concatenate([f(inputs["cls_w"]).T, f(inputs["attn_w"]).T], axis=1)),
        "biasrow": bf(brow),
        "onesrow": bf(np.ones((1, 128), np.float32)),
        "onec": np.ones((128, 1), np.float32),
        "epsc": np.full((128, 1), 1e-5, np.float32),
    }
    xTall = np.ascontiguousarray(f(inputs["x_enc"]).transpose(0, 2, 1))  # [B, CIN, L]
    xm = f(inputs["x_mark_enc"])
    per_core = []
    for c in range(NCORES):
        m = dict(shared)
        m["xT"] = np.ascontiguousarray(xTall[c * BLOC:(c + 1) * BLOC]).astype(ml_dtypes.bfloat16)
        m["xmark2"] = np.ascontiguousarray(xm[c * BLOC:(c + 1) * BLOC]).astype(ml_dtypes.bfloat16)
        per_core.append(m)
    return per_core


def kernel(**inputs) -> np.ndarray:
    from concourse.bass_utils import run_bass_kernel_spmd

    nc = _get_module()
    in_maps = _prep_inputs(inputs)
    res = run_bass_kernel_spmd(nc, in_maps, core_ids=list(range(NCORES)))
    return np.concatenate([res.results[c]["out"] for c in range(NCORES)], axis=0)


# revision 21
# speedup vs baseline: 1.2659x; 1.1408x over previous
"""Trainium2 Bass kernel for nn_Model_14998025797662 (Mamba-TimeVariant classifier).

Self-contained: hardcodes shapes. Data-parallel over batch: 16 samples ->
8 cores x 2 samples. Layout: channels-on-partitions, time-on-free.

v2: both per-core samples are packed along the free axis (T2 = 2048) so every
elementwise/scan op covers both samples in one instruction. The scan resets at
the sample boundary via a poisoned dt column (dt=+30 -> dA=exp(A*30)=0).
Engine budget: SSM scans pinned on DVE; dA exponentials on ACT; dBu/C-term
muls split DVE(bf16 2x)/Pool; the 60 state-accumulate adds run as SWDGE
accumulate-DMAs (free of engine time). B/C broadcasts via DRAM-row bounce.
Head transposes via xbar DMA-transpose. LayerNorm stats batched across the 16
time chunks to kill the serial scalar chain.
"""

import numpy as np

import concourse.bacc as bacc
import concourse.bass as bass
from concourse import mybir
from concourse.bass import ds, ts
from concourse.tile import TileContext

F32 = mybir.dt.float32
BF16 = mybir.dt.bfloat16
AF = mybir.ActivationFunctionType
OP = mybir.AluOpType
AX = mybir.AxisListType

B, L, CIN = 16, 1024, 12
DM, DS, DC, DI, DTR = 256, 16, 4, 512, 16
NCLS, NH, EK = 10, 8, 3
NCORES = 8
BLOC = B // NCORES          # 2 samples per core
T2 = BLOC * L               # 2048 combined free axis
NDT = DI // 128             # 4 d-tiles
NCH = T2 // 128             # 16 time chunks
KD = EK * CIN               # 36
PAD = DC - 1                # 3 pad cols per sample for the causal conv
EW = T2 + BLOC * PAD        # 2054 emb width


def _off(n):
    """emb col offset of 512-chunk n (pads at [0:3] and [1027:1030])."""
    return PAD + n * 512 + PAD * (n >= 2)


def _pool_dbu(s, d):
    # dBu feeds the DVE scan directly; keeping it on DVE avoids scan stalls
    return False


def _pool_term(s, d):
    # C-side terms are off the critical path (they feed SWDGE accum DMAs);
    # keep ~44 on Pool, 20 on DVE (SWDGE desc-gen also eats Pool time)
    if d == (s % 4):
        return False
    if s % 4 == 3 and d == ((s + 1) % 4):
        return False
    return True


def _patch_act_tables():
    """Bias ACT table-set selection so Exp and Ln resolve to the same set
    (avoids per-op table thrash). Idempotent."""
    import concourse.bacc as _bacc
    import concourse.hw_specs as _hw
    if getattr(_bacc, "_ant_act_tables_patched", False):
        return
    _orig = _hw.get_activation_tables

    def patched(arch):
        t = _orig(arch)
        both = None
        for name, fns in t.items():
            sn = {str(x).split(".")[-1] for x in fns}
            if "Exp" in sn and "Ln" in sn:
                both = name
                break
        if both is not None:
            for name, fns in t.items():
                if name == both:
                    continue
                fns.discard(mybir.ActivationFunctionType.Exp)
                fns.discard(mybir.ActivationFunctionType.Ln)
        return t

    _bacc.get_activation_tables = patched
    _bacc._ant_act_tables_patched = True


def _build_module():
    _patch_act_tables()
    nc = bacc.Bacc("TRN2", target_bir_lowering=False)

    def din(name, shape, dt=F32):
        return nc.dram_tensor(name, shape, dt, kind="ExternalInput")

    xT = din("xT", [BLOC, CIN, L], BF16)
    xmark2 = din("xmark2", [BLOC, L], BF16)
    tok_lhsT = din("tok_lhsT", [KD, DM], BF16)
    peT = din("peT", [DM, L])
    inWzT = din("inWzT", [DM, DI], BF16)    # z half of in_proj
    convWT = din("convWT", [DC * DM, DI], BF16)
    dcb = din("dcb", [DI, 1])
    xprojWT = din("xprojWT", [DI, DTR + 2 * DS], BF16)
    dtWT = din("dtWT", [DTR, DI], BF16)
    dtb = din("dtb", [DI, 1])
    Amat = din("Amat", [DI, DS])
    Dv = din("Dv", [DI, 1])
    WoutT = din("WoutT", [DI, DM + 1], BF16)
    lng_bc = din("lng_bc", [128, DM], BF16)
    lnb_bc = din("lnb_bc", [128, DM])
    headWT = din("headWT", [DM, NCLS + NH], BF16)
    biasrow = din("biasrow", [1, NCLS + NH], BF16)
    onesrow = din("onesrow", [1, 128], BF16)
    onec = din("onec", [128, 1])
    epsc = din("epsc", [128, 1])

    out = nc.dram_tensor("out", [BLOC, NCLS], F32, kind="ExternalOutput")
    scr_bc = nc.dram_tensor("scr_bc", [2 * DS, T2], BF16)   # rows 0:16 C, 16:32 B
    scr_am = nc.dram_tensor("scr_am", [BLOC, L], F32)
    scr_wx = nc.dram_tensor("scr_wx", [BLOC, L], F32)

    with TileContext(nc) as tc:
        with (
            tc.tile_pool(name="const", bufs=1) as cp,
            tc.tile_pool(name="persist", bufs=1) as pp,
            tc.tile_pool(name="work", bufs=2) as wp,
            tc.tile_pool(name="small", bufs=2) as sp,
            tc.tile_pool(name="psumr", bufs=3, space="PSUM") as psr,
        ):
            def cload(name, shape, src, dt=F32):
                t = cp.tile(shape, dt, name=f"c_{name}")
                nc.sync.dma_start(t[:], src)
                return t

            # inputs + first-use weights load first so stage A starts early
            tokW_sb = cload("tokW", [KD, DM], tok_lhsT[:], BF16)
            rhs36 = pp.tile([KD, T2], BF16, name="rhs36", tag="skinny", bufs=2)
            for b in range(BLOC):
                c0 = b * L
                nc.scalar.dma_start(rhs36[12:24, c0:c0 + L], xT[b, :, :])
                nc.scalar.dma_start(rhs36[0:12, c0 + 1:c0 + L], xT[b, :, 0:L - 1])
                nc.scalar.dma_start(rhs36[0:12, c0:c0 + 1], xT[b, :, 0:1])
                nc.scalar.dma_start(rhs36[24:36, c0:c0 + L - 1], xT[b, :, 1:L])
                nc.scalar.dma_start(rhs36[24:36, c0 + L - 1:c0 + L], xT[b, :, L - 1:L])
            pe_sb = []
            for m in range(2):
                # staged in the dbu work-tag ring (dead before stage B uses it)
                pt = wp.tile([128, L], F32, name=f"pe{m}", tag="dbu", bufs=2)
                nc.scalar.dma_start(pt[:], peT[ts(m, 128), :])
                pe_sb.append(pt)
            inWz_sb = [cload(f"inWz{k}", [128, DI], inWzT[ts(k, 128), :], BF16) for k in range(2)]
            convW_sb = [cload(f"cvW{k}", [128, DI], convWT[ts(k, 128), :], BF16) for k in range(8)]
            dcb_sb = [cload(f"dcb{d}", [128, 1], dcb[ts(d, 128), :]) for d in range(NDT)]
            xprojW_sb = [cload(f"xpW{d}", [128, DTR + 2 * DS], xprojWT[ts(d, 128), :], BF16) for d in range(NDT)]
            dtW_sb = cload("dtW", [DTR, DI], dtWT[:], BF16)
            dtb_sb = [cload(f"dtb{d}", [128, 1], dtb[ts(d, 128), :]) for d in range(NDT)]
            A_sb = [cload(f"A{d}", [128, DS], Amat[ts(d, 128), :]) for d in range(NDT)]
            Dv_sb = [cload(f"Dv{d}", [128, 1], Dv[ts(d, 128), :]) for d in range(NDT)]
            Wout_sb = [cload(f"Wo{d}", [128, DM + 1], WoutT[ts(d, 128), :], BF16) for d in range(NDT)]
            lng_sb = cload("lng", [128, DM], lng_bc[:], BF16)
            lnb_sb = cload("lnb", [128, DM], lnb_bc[:])
            headW_sb = [cload(f"hW{k}", [128, NCLS + NH], headWT[ts(k, 128), :], BF16) for k in range(2)]
            bias_sb = cload("biasrow", [1, NCLS + NH], biasrow[:], BF16)
            ones_sb = cload("onesrow", [1, 128], onesrow[:], BF16)
            one_sb = cload("onec", [128, 1], onec[:])
            eps_sb = cload("epsc", [128, 1], epsc[:])
            xmrow = wp.tile([BLOC, L], BF16, name="xmrow", tag="xmk", bufs=1)
            nc.sync.dma_start(xmrow[:], xmark2[:, :])

            # ======== stage A ========

            emb_sb = [pp.tile([128, EW], BF16, name=f"emb{m}", tag="embt", bufs=2) for m in range(2)]
            for m in range(2):
                nc.vector.memset(emb_sb[m][:, 0:PAD], 0.0)
                nc.vector.memset(emb_sb[m][:, PAD + L:PAD + L + PAD], 0.0)

            sz_sb = [pp.tile([128, T2], BF16, name=f"sz{d}", tag=f"sz{d}") for d in range(NDT)]
            u0_sb = [pp.tile([128, T2], BF16, name=f"u0{d}", tag=f"u0{d}") for d in range(NDT)]
            xdbl_sb = pp.tile([DTR + 2 * DS, T2], BF16, name="xdbl", tag="skinny", bufs=2)
            dt_sb = [pp.tile([128, T2], BF16, name=f"dt{d}", tag=f"dt{d}") for d in range(NDT)]

            # pass 1 (silu act-table): emb, z-silu, conv-silu per chunk
            for n in range(4):
                o = _off(n)
                # emb chunk: tok conv + positional embedding
                for m in range(2):
                    ps = psr.tile([128, 512], F32, name=f"eps{m}{n}", tag="ps512")
                    nc.tensor.matmul(ps[:], tokW_sb[:, ts(m, 128)], rhs36[:, ts(n, 512)],
                                     start=True, stop=True)
                    nc.vector.tensor_add(emb_sb[m][:, ds(o, 512)], ps[:],
                                         pe_sb[m][:, ds((n % 2) * 512, 512)])
                # z half -> silu
                for d in range(NDT):
                    ps = psr.tile([128, 512], F32, name=f"z{d}{n}", tag="ps512")
                    for k in range(2):
                        nc.tensor.matmul(ps[:], inWz_sb[k][:, ts(d, 128)],
                                         emb_sb[k][:, ds(o, 512)],
                                         start=(k == 0), stop=(k == 1))
                    nc.scalar.activation(sz_sb[d][:, ts(n, 512)], ps[:], AF.Silu)
                # fused causal conv of in_proj x-half -> silu
                for d in range(NDT):
                    ps = psr.tile([128, 512], F32, name=f"u{d}{n}", tag="ps512")
                    for k in range(8):
                        j = k // 2
                        nc.tensor.matmul(ps[:], convW_sb[k][:, ts(d, 128)],
                                         emb_sb[k % 2][:, ds(o - PAD + j, 512)],
                                         start=(k == 0), stop=(k == 7))
                    nc.scalar.activation(u0_sb[d][:, ts(n, 512)], ps[:], AF.Silu,
                                         bias=dcb_sb[d][:, 0:1])
            # pass 2 (no act table): x_proj -> x_dblT
            for n in range(4):
                ps = psr.tile([DTR + 2 * DS, 512], F32, name=f"xd{n}", tag="ps512")
                for k in range(NDT):
                    nc.tensor.matmul(ps[:], xprojW_sb[k][:], u0_sb[k][:, ts(n, 512)],
                                     start=(k == 0), stop=(k == NDT - 1))
                nc.scalar.copy(xdbl_sb[:, ts(n, 512)], ps[:])
            # pass 3 (exp/ln act-table): dt = softplus
            for n in range(4):
                for d in range(NDT):
                    ps = psr.tile([128, 512], F32, name=f"dtp{d}{n}", tag="ps512")
                    nc.tensor.matmul(ps[:], dtW_sb[:, ts(d, 128)], xdbl_sb[0:DTR, ts(n, 512)],
                                     start=True, stop=True)
                    esp = psr.tile([128, 512], F32, name=f"esp{d}{n}", tag="ps512")
                    nc.scalar.activation(esp[:], ps[:], AF.Exp, bias=dtb_sb[d][:, 0:1])
                    nc.scalar.activation(dt_sb[d][:, ts(n, 512)], esp[:], AF.Ln,
                                         bias=one_sb[:, 0:1])

            # stage B/C broadcast rows to DRAM (single wide write)
            nc.sync.dma_start(scr_bc[:, :], xdbl_sb[DTR:DTR + 2 * DS, :])

            # w = dt*u, then poison dt col at the sample boundary so dA goes
            # to 0 there (scan state reset)
            wT_sb = [pp.tile([128, T2], BF16, name=f"w{d}", tag=f"w{d}") for d in range(NDT)]
            for d in range(NDT):
                nc.vector.tensor_mul(wT_sb[d][:], dt_sb[d][:], u0_sb[d][:])
            for d in range(NDT):
                nc.vector.memset(dt_sb[d][:, L:L + 1], 30.0)

            # ======== stage B: 16 SSM states ========
            acc = [pp.tile([128, T2], BF16, name=f"acc{d}", tag=f"acc{d}") for d in range(NDT)]
            for s in range(DS):
                bbc = wp.tile([128, T2], BF16, name=f"bbc{s}", tag="bbc", bufs=2)
                nc.sync.dma_start(bbc[:], scr_bc[DS + s:DS + s + 1, :].to_broadcast((128, T2)))
                cbc = wp.tile([128, T2], BF16, name=f"cbc{s}", tag="cbc", bufs=2)
                nc.sync.dma_start(cbc[:], scr_bc[s:s + 1, :].to_broadcast((128, T2)))
                dAs, dBus, hs = [], [], []
                for d in range(NDT):
                    dA = wp.tile([128, T2], F32, name=f"dA{s}{d}", tag="dA", bufs=2)
                    nc.scalar.activation(dA[:], dt_sb[d][:], AF.Exp, scale=A_sb[d][:, s:s + 1])
                    dAs.append(dA)
                for d in range(NDT):
                    dBu = wp.tile([128, T2], BF16, name=f"dBu{s}{d}", tag="dbu", bufs=2)
                    eng = nc.gpsimd if _pool_dbu(s, d) else nc.vector
                    eng.tensor_mul(dBu[:], wT_sb[d][:], bbc[:])
                    dBus.append(dBu)
                for d in range(NDT):
                    h = wp.tile([128, T2], BF16, name=f"h{s}{d}", tag="hh", bufs=4)
                    nc.vector.tensor_tensor_scan(h[:], dAs[d][:], dBus[d][:], 0.0,
                                                 op0=OP.mult, op1=OP.add)
                    hs.append(h)
                for d in range(NDT):
                    eng = nc.gpsimd if _pool_term(s, d) else nc.vector
                    if s == 0:
                        eng.tensor_mul(acc[d][:], hs[d][:], cbc[:])
                    else:
                        term = wp.tile([128, T2], BF16, name=f"term{s}{d}", tag="term", bufs=3)
                        eng.tensor_mul(term[:], hs[d][:], cbc[:])
                        nc.gpsimd.dma_start(acc[d][:], term[:], accum_op=OP.add)

            # ytot = (acc + u*D) * sz, stored back into the w tiles
            for d in range(NDT):
                t1 = wp.tile([128, T2], BF16, name=f"yt1{d}", tag="dbu", bufs=2)
                nc.vector.scalar_tensor_tensor(t1[:], u0_sb[d][:], Dv_sb[d][:, 0:1],
                                               acc[d][:], op0=OP.mult, op1=OP.add)
                nc.vector.tensor_mul(wT_sb[d][:], t1[:], sz_sb[d][:])

            # ======== stage C ========
            ssum_all = sp.tile([128, NCH], F32, name="ssum_all", tag="ssum", bufs=1)
            sqs_all = sp.tile([128, NCH], F32, name="sqs_all", tag="sqs", bufs=1)
            mos_half = [pp.tile([128, T2], BF16, name=f"mosh{i}", tag="skinny", bufs=2)
                        for i in range(2)]
            mosb = [mos_half[t // 8][:, ds(256 * (t % 8), DM)] for t in range(NCH)]
            for t in range(NCH):
                mp = psr.tile([128, DM + 1], F32, name=f"mo{t}", tag="MO", bufs=3)
                for d in range(NDT):
                    nc.tensor.matmul(mp[:], wT_sb[d][:, ts(t, 128)], Wout_sb[d][:],
                                     start=(d == 0), stop=(d == NDT - 1))
                nc.scalar.copy(mosb[t], mp[:, 0:DM])
                nc.vector.tensor_copy(ssum_all[:, t:t + 1], mp[:, DM:DM + 1])
                sq = wp.tile([128, DM], BF16, name=f"sq{t}", tag="xn", bufs=2)
                nc.vector.scalar_tensor_tensor(sq[:], mosb[t], 1.0, mosb[t],
                                               op0=OP.mult, op1=OP.mult,
                                               accum_out=sqs_all[:, t:t + 1])

            # batched LN stats over all 16 chunks
            mun = sp.tile([128, NCH], F32, name="mun", tag="mun")
            nc.vector.tensor_scalar_mul(mun[:], ssum_all[:], -1.0 / DM)
            m2t = sp.tile([128, NCH], F32, name="m2t", tag="m2t")
            nc.vector.tensor_scalar_mul(m2t[:], sqs_all[:], 1.0 / DM)
            msq = sp.tile([128, NCH], F32, name="msq", tag="msq")
            nc.vector.tensor_mul(msq[:], mun[:], mun[:])
            var = sp.tile([128, NCH], F32, name="var", tag="var")
            nc.vector.tensor_sub(var[:], m2t[:], msq[:])
            lnv = sp.tile([128, NCH], F32, name="lnv", tag="lnv")
            nc.scalar.activation(lnv[:], var[:], AF.Ln, bias=eps_sb[:, 0:1])
            rstd = sp.tile([128, NCH], F32, name="rstd", tag="rstd")
            nc.scalar.activation(rstd[:], lnv[:], AF.Exp, scale=-0.5)
            nmr = sp.tile([128, NCH], F32, name="nmr", tag="nmr")
            nc.vector.tensor_mul(nmr[:], mun[:], rstd[:])

            lg_all = []
            for t in range(NCH):
                xn = wp.tile([128, DM], BF16, name=f"xn{t}", tag="xn", bufs=2)
                nc.scalar.activation(xn[:], mosb[t], AF.Identity,
                                     bias=nmr[:, t:t + 1], scale=rstd[:, t:t + 1])
                t1 = wp.tile([128, DM], BF16, name=f"t1{t}", tag="t1", bufs=2)
                nc.vector.tensor_mul(t1[:], xn[:], lng_sb[:])
                t2 = wp.tile([128, DM], BF16, name=f"t2{t}", tag="t2", bufs=2)
                nc.gpsimd.tensor_add(t2[:], t1[:], lnb_sb[:])
                mam = wp.tile([128, DM], BF16, name=f"mam{t}", tag="mam", bufs=2)
                nc.scalar.activation(mam[:], t2[:], AF.Silu)
                moT = [wp.tile([128, 128], BF16, name=f"moT{t}{m}", tag="moT", bufs=4)
                       for m in range(2)]
                nc.sync.dma_start(moT[0][:], mam[:, 0:128], transpose=True)
                nc.scalar.dma_start(moT[1][:], mam[:, 128:256], transpose=True)
                hd = psr.tile([128, NCLS + NH], F32, name=f"hd{t}", tag="HD", bufs=2)
                for k in range(2):
                    nc.tensor.matmul(hd[:], moT[k][:], headW_sb[k][:],
                                     start=(k == 0), stop=False)
                nc.tensor.matmul(hd[:], ones_sb[:], bias_sb[:], start=False, stop=True)
                lg = pp.tile([128, NCLS], F32, name=f"lg{t}", tag=f"lg{t}")
                nc.vector.tensor_copy(lg[:], hd[:, 0:NCLS])
                lg_all.append(lg)
                am = sp.tile([128, 1], F32, name=f"am{t}", tag="am", bufs=2)
                nc.vector.reduce_max(am[:], hd[:, NCLS:NCLS + NH], axis=AX.X)
                nc.sync.dma_start(scr_am[t // 8, ds(128 * (t % 8), 128)], am[:])

            # tail: per-sample softmax over time (sample = partition)
            row = wp.tile([BLOC, L], F32, name="row_am", tag="term", bufs=3)
            nc.sync.dma_start(row[:], scr_am[:, :])
            mx = sp.tile([BLOC, 1], F32, name="mx", tag="mx")
            nc.vector.reduce_max(mx[:], row[:], axis=AX.X)
            nmx = sp.tile([BLOC, 1], F32, name="nmx", tag="nmx")
            nc.vector.tensor_scalar_mul(nmx[:], mx[:], -1.0)
            ex = wp.tile([BLOC, L], F32, name="ex", tag="term", bufs=3)
            esum = sp.tile([BLOC, 1], F32, name="esum", tag="esum")
            nc.scalar.activation(ex[:], row[:], AF.Exp, bias=nmx[:, 0:1],
                                 accum_out=esum[:])
            rec = sp.tile([BLOC, 1], F32, name="rec", tag="rec")
            nc.vector.reciprocal(rec[:], esum[:])
            wx = wp.tile([BLOC, L], F32, name="wx", tag="term", bufs=3)
            nc.vector.scalar_tensor_tensor(wx[:], ex[:], rec[:, 0:1], xmrow[:],
                                           op0=OP.mult, op1=OP.mult)
            for b in range(BLOC):
                nc.sync.dma_start(scr_wx[b:b + 1, :], wx[b:b + 1, :])

            wxc = []
            for t in range(NCH):
                wc = sp.tile([128, 1], F32, name=f"wxc{t}", tag=f"wxc{t}")
                nc.sync.dma_start(wc[:], scr_wx[t // 8, ds(128 * (t % 8), 128)])
                wxc.append(wc)
            for b in range(BLOC):
                ops = psr.tile([NCLS, 1], F32, name=f"ops{b}", tag="HD", bufs=2)
                for i in range(8):
                    t = 8 * b + i
                    nc.tensor.matmul(ops[:], lg_all[t][:], wxc[t][:],
                                     start=(i == 0), stop=(i == 7))
                oc = sp.tile([NCLS, 1], F32, name=f"oc{b}", tag=f"oc{b}")
                nc.vector.tensor_copy(oc[:], ops[:])
                nc.sync.dma_start(out[b, :], oc[:])

    nc.finalize()
    return nc


_NC_CACHE = None


def _get_module():
    global _NC_CACHE
    if _NC_CACHE is None:
        _NC_CACHE = _build_module()
    return _NC_CACHE


def _pos_emb_T():
    pos = np.arange(L, dtype=np.float32)[:, None]
    div = np.exp(np.arange(0, DM, 2, dtype=np.float32) * (-np.log(10000.0) / DM))
    pe = np.zeros((L, DM), np.float32)
    pe[:, 0::2] = np.sin(pos * div)
    pe[:, 1::2] = np.cos(pos * div)
    return pe.T.copy()


def _prep_inputs(inputs):
    import ml_dtypes
    f = lambda x: np.ascontiguousarray(np.asarray(x, dtype=np.float32))
    bf = lambda x: np.ascontiguousarray(x).astype(ml_dtypes.bfloat16)
    tokW = f(inputs["tok_conv_w"])                        # [DM, CIN, EK]
    inW = f(inputs["in_proj_w"])                          # [2DI, DM]
    cvw = f(inputs["dconv_w"])[:, 0, :]                   # [DI, DC]
    # convWT[(j,m), d] = in_proj_w[d, m] * dconv_w[d, j]
    convWT = (inW[:DI][None, :, :] * cvw.T[:, :, None]).transpose(0, 2, 1)  # [DC, DM, DI]
    convWT = np.ascontiguousarray(convWT.reshape(DC * DM, DI))
    attnb = f(inputs["attn_b"])
    brow = np.zeros((1, NCLS + NH), np.float32)
    brow[0, NCLS:] = attnb
    shared = {
        "tok_lhsT": bf(np.transpose(tokW, (2, 1, 0)).reshape(KD, DM)),
        "peT": _pos_emb_T(),
        "inWzT": bf(inW[DI:].T),
        "convWT": bf(convWT),
        "dcb": f(inputs["dconv_b"]).reshape(DI, 1),
        "xprojWT": bf(f(inputs["x_proj_w"]).T[:, list(range(DTR)) + list(range(DTR + DS, DTR + 2 * DS)) + list(range(DTR, DTR + DS))]),
        "dtWT": bf(f(inputs["dt_proj_w"]).T),
        "dtb": f(inputs["dt_proj_b"]).reshape(DI, 1),
        "Amat": (-np.exp(f(inputs["A_log"]))).astype(np.float32),
        "Dv": f(inputs["Dvec"]).reshape(DI, 1),
        "WoutT": bf(np.concatenate([f(inputs["out_proj_w"]).T,
                                    f(inputs["out_proj_w"]).T.sum(axis=1, keepdims=True)], axis=1)),
        "lng_bc": bf(np.broadcast_to(f(inputs["ln_g"]), (128, DM)).copy()),
        "lnb_bc": np.broadcast_to(f(inputs["ln_b"]), (128, DM)).copy(),
        "headWT": bf(np.concatenate([f(inputs["cls_w"]).T, f(inputs["attn_w"]).T], axis=1)),
        "biasrow": bf(brow),
        "onesrow": bf(np.ones((1, 128), np.float32)),
        "onec": np.ones((128, 1), np.float32),
        "epsc": np.full((128, 1), 1e-5, np.float32),
    }
    xTall = np.ascontiguousarray(f(inputs["x_enc"]).transpose(0, 2, 1))  # [B, CIN, L]
    xm = f(inputs["x_mark_enc"])
    per_core = []
    for c in range(NCORES):
        m = dict(shared)
        m["xT"] = np.ascontiguousarray(xTall[c * BLOC:(c + 1) * BLOC]).astype(ml_dtypes.bfloat16)
        m["xmark2"] = np.ascontiguousarray(xm[c * BLOC:(c + 1) * BLOC]).astype(ml_dtypes.bfloat16)
        per_core.append(m)
    return per_core


def kernel(**inputs) -> np.ndarray:
    from concourse.bass_utils import run_bass_kernel_spmd

    nc = _get_module()
    in_maps = _prep_inputs(inputs)
    res = run_bass_kernel_spmd(nc, in_maps, core_ids=list(range(NCORES)))
    return np.concatenate([res.results[c]["out"] for c in range(NCORES)], axis=0)


# revision 22
# speedup vs baseline: 1.2929x; 1.0213x over previous
"""Trainium2 Bass kernel for nn_Model_14998025797662 (Mamba-TimeVariant classifier).

Self-contained: hardcodes shapes. Data-parallel over batch: 16 samples ->
8 cores x 2 samples. Layout: channels-on-partitions, time-on-free.

v2: both per-core samples are packed along the free axis (T2 = 2048) so every
elementwise/scan op covers both samples in one instruction. The scan resets at
the sample boundary via a poisoned dt column (dt=+30 -> dA=exp(A*30)=0).
Engine budget: SSM scans pinned on DVE; dA exponentials on ACT; dBu/C-term
muls split DVE(bf16 2x)/Pool; the 60 state-accumulate adds run as SWDGE
accumulate-DMAs (free of engine time). B/C broadcasts via DRAM-row bounce.
Head transposes via xbar DMA-transpose. LayerNorm stats batched across the 16
time chunks to kill the serial scalar chain.
"""

import numpy as np

import concourse.bacc as bacc
import concourse.bass as bass
from concourse import mybir
from concourse.bass import ds, ts
from concourse.tile import TileContext

F32 = mybir.dt.float32
BF16 = mybir.dt.bfloat16
AF = mybir.ActivationFunctionType
OP = mybir.AluOpType
AX = mybir.AxisListType

B, L, CIN = 16, 1024, 12
DM, DS, DC, DI, DTR = 256, 16, 4, 512, 16
NCLS, NH, EK = 10, 8, 3
NCORES = 8
BLOC = B // NCORES          # 2 samples per core
T2 = BLOC * L               # 2048 combined free axis
NDT = DI // 128             # 4 d-tiles
NCH = T2 // 128             # 16 time chunks
KD = EK * CIN               # 36
PAD = DC - 1                # 3 pad cols per sample for the causal conv
EW = T2 + BLOC * PAD        # 2054 emb width


def _off(n):
    """emb col offset of 512-chunk n (pads at [0:3] and [1027:1030])."""
    return PAD + n * 512 + PAD * (n >= 2)


def _pool_dbu(s, d):
    # dBu feeds the DVE scan directly; keeping it on DVE avoids scan stalls
    return False


def _pool_term(s, d):
    # C-side terms are off the critical path (they feed SWDGE accum DMAs);
    # keep ~44 on Pool, 20 on DVE (SWDGE desc-gen also eats Pool time)
    if d == (s % 4):
        return False
    if s % 4 == 3 and d == ((s + 1) % 4):
        return False
    return True


def _patch_act_tables():
    """Bias ACT table-set selection so Exp and Ln resolve to the same set
    (avoids per-op table thrash). Idempotent."""
    import concourse.bacc as _bacc
    import concourse.hw_specs as _hw
    if getattr(_bacc, "_ant_act_tables_patched", False):
        return
    _orig = _hw.get_activation_tables

    def patched(arch):
        t = _orig(arch)
        both = None
        for name, fns in t.items():
            sn = {str(x).split(".")[-1] for x in fns}
            if "Exp" in sn and "Ln" in sn:
                both = name
                break
        if both is not None:
            for name, fns in t.items():
                if name == both:
                    continue
                fns.discard(mybir.ActivationFunctionType.Exp)
                fns.discard(mybir.ActivationFunctionType.Ln)
        return t

    _bacc.get_activation_tables = patched
    _bacc._ant_act_tables_patched = True


def _build_module():
    _patch_act_tables()
    nc = bacc.Bacc("TRN2", target_bir_lowering=False)

    def din(name, shape, dt=F32):
        return nc.dram_tensor(name, shape, dt, kind="ExternalInput")

    xT = din("xT", [BLOC, CIN, L], BF16)
    xmark2 = din("xmark2", [BLOC, L], BF16)
    tok_lhsT = din("tok_lhsT", [KD, DM], BF16)
    peT = din("peT", [DM, L])
    inWzT = din("inWzT", [DM, DI], BF16)    # z half of in_proj
    convWT = din("convWT", [DC * DM, DI], BF16)
    dcb = din("dcb", [DI, 1])
    xprojWT = din("xprojWT", [DI, DTR + 2 * DS], BF16)
    dtWT = din("dtWT", [DTR, DI], BF16)
    dtb = din("dtb", [DI, 1])
    Amat = din("Amat", [DI, DS])
    Dv = din("Dv", [DI, 1])
    WoutT = din("WoutT", [DI, DM + 1], BF16)
    lng_bc = din("lng_bc", [128, DM], BF16)
    lnb_bc = din("lnb_bc", [128, DM])
    headWT = din("headWT", [DM, NCLS + NH], BF16)
    biasrow = din("biasrow", [1, NCLS + NH], BF16)
    onesrow = din("onesrow", [1, 128], BF16)
    onec = din("onec", [128, 1])
    epsc = din("epsc", [128, 1])

    out = nc.dram_tensor("out", [BLOC, NCLS], F32, kind="ExternalOutput")
    scr_bc = nc.dram_tensor("scr_bc", [2 * DS, T2], BF16)   # rows 0:16 C, 16:32 B

    with TileContext(nc) as tc:
        with (
            tc.tile_pool(name="const", bufs=1) as cp,
            tc.tile_pool(name="persist", bufs=1) as pp,
            tc.tile_pool(name="work", bufs=2) as wp,
            tc.tile_pool(name="small", bufs=2) as sp,
            tc.tile_pool(name="psumr", bufs=3, space="PSUM") as psr,
        ):
            def cload(name, shape, src, dt=F32):
                t = cp.tile(shape, dt, name=f"c_{name}")
                nc.sync.dma_start(t[:], src)
                return t

            # inputs + first-use weights load first so stage A starts early
            tokW_sb = cload("tokW", [KD, DM], tok_lhsT[:], BF16)
            rhs36 = pp.tile([KD, T2], BF16, name="rhs36", tag="skinny", bufs=2)
            for b in range(BLOC):
                c0 = b * L
                nc.scalar.dma_start(rhs36[12:24, c0:c0 + L], xT[b, :, :])
                nc.scalar.dma_start(rhs36[0:12, c0 + 1:c0 + L], xT[b, :, 0:L - 1])
                nc.scalar.dma_start(rhs36[0:12, c0:c0 + 1], xT[b, :, 0:1])
                nc.scalar.dma_start(rhs36[24:36, c0:c0 + L - 1], xT[b, :, 1:L])
                nc.scalar.dma_start(rhs36[24:36, c0 + L - 1:c0 + L], xT[b, :, L - 1:L])
            pe_sb = []
            for m in range(2):
                # staged in the dbu work-tag ring (dead before stage B uses it)
                pt = wp.tile([128, L], F32, name=f"pe{m}", tag="dbu", bufs=2)
                nc.sync.dma_start(pt[:], peT[ts(m, 128), :])
                pe_sb.append(pt)
            inWz_sb = [cload(f"inWz{k}", [128, DI], inWzT[ts(k, 128), :], BF16) for k in range(2)]
            convW_sb = [cload(f"cvW{k}", [128, DI], convWT[ts(k, 128), :], BF16) for k in range(8)]
            dcb_sb = [cload(f"dcb{d}", [128, 1], dcb[ts(d, 128), :]) for d in range(NDT)]
            xprojW_sb = [cload(f"xpW{d}", [128, DTR + 2 * DS], xprojWT[ts(d, 128), :], BF16) for d in range(NDT)]
            dtW_sb = cload("dtW", [DTR, DI], dtWT[:], BF16)
            dtb_sb = [cload(f"dtb{d}", [128, 1], dtb[ts(d, 128), :]) for d in range(NDT)]
            A_sb = [cload(f"A{d}", [128, DS], Amat[ts(d, 128), :]) for d in range(NDT)]
            Dv_sb = [cload(f"Dv{d}", [128, 1], Dv[ts(d, 128), :]) for d in range(NDT)]
            Wout_sb = [cload(f"Wo{d}", [128, DM + 1], WoutT[ts(d, 128), :], BF16) for d in range(NDT)]
            lng_sb = cload("lng", [128, DM], lng_bc[:], BF16)
            lnb_sb = cload("lnb", [128, DM], lnb_bc[:])
            headW_sb = [cload(f"hW{k}", [128, NCLS + NH], headWT[ts(k, 128), :], BF16) for k in range(2)]
            bias_sb = cload("biasrow", [1, NCLS + NH], biasrow[:], BF16)
            ones_sb = cload("onesrow", [1, 128], onesrow[:], BF16)
            one_sb = cload("onec", [128, 1], onec[:])
            eps_sb = cload("epsc", [128, 1], epsc[:])
            xmrow = wp.tile([BLOC, L], BF16, name="xmrow", tag="xmk", bufs=1)
            nc.sync.dma_start(xmrow[:], xmark2[:, :])

            # ======== stage A ========

            emb_sb = [pp.tile([128, EW], BF16, name=f"emb{m}", tag="embt", bufs=2) for m in range(2)]
            for m in range(2):
                nc.vector.memset(emb_sb[m][:, 0:PAD], 0.0)
                nc.vector.memset(emb_sb[m][:, PAD + L:PAD + L + PAD], 0.0)

            sz_sb = [pp.tile([128, T2], BF16, name=f"sz{d}", tag=f"sz{d}") for d in range(NDT)]
            u0_sb = [pp.tile([128, T2], BF16, name=f"u0{d}", tag=f"u0{d}") for d in range(NDT)]
            xdbl_sb = pp.tile([DTR + 2 * DS, T2], BF16, name="xdbl", tag="skinny", bufs=2)
            dt_sb = [pp.tile([128, T2], BF16, name=f"dt{d}", tag=f"dt{d}") for d in range(NDT)]

            # pass 1 (silu act-table): emb, z-silu, conv-silu per chunk
            for n in range(4):
                o = _off(n)
                # emb chunk: tok conv + positional embedding
                for m in range(2):
                    ps = psr.tile([128, 512], F32, name=f"eps{m}{n}", tag="ps512")
                    nc.tensor.matmul(ps[:], tokW_sb[:, ts(m, 128)], rhs36[:, ts(n, 512)],
                                     start=True, stop=True)
                    nc.vector.tensor_add(emb_sb[m][:, ds(o, 512)], ps[:],
                                         pe_sb[m][:, ds((n % 2) * 512, 512)])
                # z half -> silu
                for d in range(NDT):
                    ps = psr.tile([128, 512], F32, name=f"z{d}{n}", tag="ps512")
                    for k in range(2):
                        nc.tensor.matmul(ps[:], inWz_sb[k][:, ts(d, 128)],
                                         emb_sb[k][:, ds(o, 512)],
                                         start=(k == 0), stop=(k == 1))
                    nc.scalar.activation(sz_sb[d][:, ts(n, 512)], ps[:], AF.Silu)
                # fused causal conv of in_proj x-half -> silu
                for d in range(NDT):
                    ps = psr.tile([128, 512], F32, name=f"u{d}{n}", tag="ps512")
                    for k in range(8):
                        j = k // 2
                        nc.tensor.matmul(ps[:], convW_sb[k][:, ts(d, 128)],
                                         emb_sb[k % 2][:, ds(o - PAD + j, 512)],
                                         start=(k == 0), stop=(k == 7))
                    nc.scalar.activation(u0_sb[d][:, ts(n, 512)], ps[:], AF.Silu,
                                         bias=dcb_sb[d][:, 0:1])
            # pass 2 (no act table): x_proj -> x_dblT
            for n in range(4):
                ps = psr.tile([DTR + 2 * DS, 512], F32, name=f"xd{n}", tag="ps512")
                for k in range(NDT):
                    nc.tensor.matmul(ps[:], xprojW_sb[k][:], u0_sb[k][:, ts(n, 512)],
                                     start=(k == 0), stop=(k == NDT - 1))
                nc.vector.tensor_copy(xdbl_sb[:, ts(n, 512)], ps[:])
            # stage B/C broadcast rows to DRAM (single wide write)
            nc.sync.dma_start(scr_bc[:, :], xdbl_sb[DTR:DTR + 2 * DS, :])

            # pass 3 (exp/ln act-table): dt = softplus, d-outer so stage B can
            # start on d=0 while later d-tiles still compute. w = dt*u, then
            # poison the dt col at the sample boundary so dA goes to 0 there
            # (scan state reset).
            wT_sb = [pp.tile([128, T2], BF16, name=f"w{d}", tag=f"w{d}") for d in range(NDT)]
            for d in range(NDT):
                for n in range(4):
                    ps = psr.tile([128, 512], F32, name=f"dtp{d}{n}", tag="ps512")
                    nc.tensor.matmul(ps[:], dtW_sb[:, ts(d, 128)], xdbl_sb[0:DTR, ts(n, 512)],
                                     start=True, stop=True)
                    esp = psr.tile([128, 512], F32, name=f"esp{d}{n}", tag="ps512")
                    nc.scalar.activation(esp[:], ps[:], AF.Exp, bias=dtb_sb[d][:, 0:1])
                    nc.scalar.activation(dt_sb[d][:, ts(n, 512)], esp[:], AF.Ln,
                                         bias=one_sb[:, 0:1])
                nc.vector.tensor_mul(wT_sb[d][:], dt_sb[d][:], u0_sb[d][:])
                nc.vector.memset(dt_sb[d][:, L:L + 1], 30.0)

            # ======== stage B: 16 SSM states ========
            acc = [pp.tile([128, T2], BF16, name=f"acc{d}", tag=f"acc{d}") for d in range(NDT)]
            for s in range(DS):
                bbc = wp.tile([128, T2], BF16, name=f"bbc{s}", tag="bbc", bufs=2)
                nc.sync.dma_start(bbc[:], scr_bc[DS + s:DS + s + 1, :].to_broadcast((128, T2)))
                cbc = wp.tile([128, T2], BF16, name=f"cbc{s}", tag="cbc", bufs=2)
                nc.sync.dma_start(cbc[:], scr_bc[s:s + 1, :].to_broadcast((128, T2)))
                dAs, dBus, hs = [], [], []
                for d in range(NDT):
                    dA = wp.tile([128, T2], F32, name=f"dA{s}{d}", tag="dA", bufs=2)
                    nc.scalar.activation(dA[:], dt_sb[d][:], AF.Exp, scale=A_sb[d][:, s:s + 1])
                    dAs.append(dA)
                for d in range(NDT):
                    dBu = wp.tile([128, T2], BF16, name=f"dBu{s}{d}", tag="dbu", bufs=2)
                    eng = nc.gpsimd if _pool_dbu(s, d) else nc.vector
                    eng.tensor_mul(dBu[:], wT_sb[d][:], bbc[:])
                    dBus.append(dBu)
                for d in range(NDT):
                    h = wp.tile([128, T2], BF16, name=f"h{s}{d}", tag="hh", bufs=4)
                    nc.vector.tensor_tensor_scan(h[:], dAs[d][:], dBus[d][:], 0.0,
                                                 op0=OP.mult, op1=OP.add)
                    hs.append(h)
                for d in range(NDT):
                    eng = nc.gpsimd if _pool_term(s, d) else nc.vector
                    if s == 0:
                        eng.tensor_mul(acc[d][:], hs[d][:], cbc[:])
                    else:
                        term = wp.tile([128, T2], BF16, name=f"term{s}{d}", tag="term", bufs=3)
                        eng.tensor_mul(term[:], hs[d][:], cbc[:])
                        nc.gpsimd.dma_start(acc[d][:], term[:], accum_op=OP.add)

            # ytot = (acc + u*D) * sz, stored back into the w tiles
            for d in range(NDT):
                t1 = wp.tile([128, T2], BF16, name=f"yt1{d}", tag="dbu", bufs=2)
                nc.vector.scalar_tensor_tensor(t1[:], u0_sb[d][:], Dv_sb[d][:, 0:1],
                                               acc[d][:], op0=OP.mult, op1=OP.add)
                nc.vector.tensor_mul(wT_sb[d][:], t1[:], sz_sb[d][:])

            # ======== stage C ========
            ssum_all = sp.tile([128, NCH], F32, name="ssum_all", tag="ssum", bufs=1)
            sqs_all = sp.tile([128, NCH], F32, name="sqs_all", tag="sqs", bufs=1)
            mos_half = [pp.tile([128, T2], BF16, name=f"mosh{i}", tag="skinny", bufs=2)
                        for i in range(2)]
            mosb = [mos_half[t // 8][:, ds(256 * (t % 8), DM)] for t in range(NCH)]
            for t in range(NCH):
                mp = psr.tile([128, DM + 1], F32, name=f"mo{t}", tag="MO", bufs=3)
                for d in range(NDT):
                    nc.tensor.matmul(mp[:], wT_sb[d][:, ts(t, 128)], Wout_sb[d][:],
                                     start=(d == 0), stop=(d == NDT - 1))
                nc.scalar.copy(mosb[t], mp[:, 0:DM])
                nc.vector.tensor_copy(ssum_all[:, t:t + 1], mp[:, DM:DM + 1])
                sq = wp.tile([128, DM], BF16, name=f"sq{t}", tag="xn", bufs=3)
                nc.vector.scalar_tensor_tensor(sq[:], mosb[t], 1.0, mosb[t],
                                               op0=OP.mult, op1=OP.mult,
                                               accum_out=sqs_all[:, t:t + 1])

            # batched LN stats over all 16 chunks
            mun = sp.tile([128, NCH], F32, name="mun", tag="mun")
            nc.vector.tensor_scalar_mul(mun[:], ssum_all[:], -1.0 / DM)
            m2t = sp.tile([128, NCH], F32, name="m2t", tag="m2t")
            nc.vector.tensor_scalar_mul(m2t[:], sqs_all[:], 1.0 / DM)
            msq = sp.tile([128, NCH], F32, name="msq", tag="msq")
            nc.vector.tensor_mul(msq[:], mun[:], mun[:])
            var = sp.tile([128, NCH], F32, name="var", tag="var")
            nc.vector.tensor_sub(var[:], m2t[:], msq[:])
            lnv = sp.tile([128, NCH], F32, name="lnv", tag="lnv")
            nc.scalar.activation(lnv[:], var[:], AF.Ln, bias=eps_sb[:, 0:1])
            rstd = sp.tile([128, NCH], F32, name="rstd", tag="rstd")
            nc.scalar.activation(rstd[:], lnv[:], AF.Exp, scale=-0.5)
            nmr = sp.tile([128, NCH], F32, name="nmr", tag="nmr")
            nc.vector.tensor_mul(nmr[:], mun[:], rstd[:])

            row = wp.tile([BLOC, L], F32, name="row_am", tag="term", bufs=3)
            lg_all = []
            for t in range(NCH):
                xn = wp.tile([128, DM], BF16, name=f"xn{t}", tag="xn", bufs=3)
                nc.scalar.activation(xn[:], mosb[t], AF.Identity,
                                     bias=nmr[:, t:t + 1], scale=rstd[:, t:t + 1])
                t1 = wp.tile([128, DM], BF16, name=f"t1{t}", tag="t1", bufs=3)
                nc.vector.tensor_mul(t1[:], xn[:], lng_sb[:])
                t2 = wp.tile([128, DM], BF16, name=f"t2{t}", tag="t2", bufs=3)
                nc.gpsimd.tensor_add(t2[:], t1[:], lnb_sb[:])
                mam = wp.tile([128, DM], BF16, name=f"mam{t}", tag="mam", bufs=3)
                nc.scalar.activation(mam[:], t2[:], AF.Silu)
                moT = [wp.tile([128, 128], BF16, name=f"moT{t}{m}", tag="moT", bufs=4)
                       for m in range(2)]
                nc.sync.dma_start(moT[0][:], mam[:, 0:128], transpose=True)
                nc.scalar.dma_start(moT[1][:], mam[:, 128:256], transpose=True)
                hd = psr.tile([128, NCLS + NH], F32, name=f"hd{t}", tag="HD", bufs=2)
                for k in range(2):
                    nc.tensor.matmul(hd[:], moT[k][:], headW_sb[k][:],
                                     start=(k == 0), stop=False)
                nc.tensor.matmul(hd[:], ones_sb[:], bias_sb[:], start=False, stop=True)
                lg = pp.tile([128, NCLS], F32, name=f"lg{t}", tag=f"lg{t}")
                nc.vector.tensor_copy(lg[:], hd[:, 0:NCLS])
                lg_all.append(lg)
                am = sp.tile([128, 1], F32, name=f"am{t}", tag="am", bufs=2)
                nc.vector.reduce_max(am[:], hd[:, NCLS:NCLS + NH], axis=AX.X)
                q = nc.sync if t % 2 == 0 else nc.scalar
                q.dma_start(row[t // 8:t // 8 + 1, ds(128 * (t % 8), 128)], am[:])

            # tail: per-sample softmax over time (sample = partition)
            mx = sp.tile([BLOC, 1], F32, name="mx", tag="mx")
            nc.vector.reduce_max(mx[:], row[:], axis=AX.X)
            nmx = sp.tile([BLOC, 1], F32, name="nmx", tag="nmx")
            nc.vector.tensor_scalar_mul(nmx[:], mx[:], -1.0)
            ex = wp.tile([BLOC, L], F32, name="ex", tag="term", bufs=3)
            esum = sp.tile([BLOC, 1], F32, name="esum", tag="esum")
            nc.scalar.activation(ex[:], row[:], AF.Exp, bias=nmx[:, 0:1],
                                 accum_out=esum[:])
            rec = sp.tile([BLOC, 1], F32, name="rec", tag="rec")
            nc.vector.reciprocal(rec[:], esum[:])
            wx = wp.tile([BLOC, L], F32, name="wx", tag="term", bufs=3)
            nc.vector.scalar_tensor_tensor(wx[:], ex[:], rec[:, 0:1], xmrow[:],
                                           op0=OP.mult, op1=OP.mult)
            wxc = []
            for t in range(NCH):
                wc = sp.tile([128, 1], F32, name=f"wxc{t}", tag=f"wxc{t}")
                q = nc.sync if t % 2 == 0 else nc.scalar
                q.dma_start(wc[:], wx[t // 8:t // 8 + 1, ds(128 * (t % 8), 128)])
                wxc.append(wc)
            for b in range(BLOC):
                ops = psr.tile([NCLS, 1], F32, name=f"ops{b}", tag="HD", bufs=2)
                for i in range(8):
                    t = 8 * b + i
                    nc.tensor.matmul(ops[:], lg_all[t][:], wxc[t][:],
                                     start=(i == 0), stop=(i == 7))
                oc = sp.tile([NCLS, 1], F32, name=f"oc{b}", tag=f"oc{b}")
                nc.vector.tensor_copy(oc[:], ops[:])
                nc.sync.dma_start(out[b, :], oc[:])

    nc.finalize()
    return nc


_NC_CACHE = None


def _get_module():
    global _NC_CACHE
    if _NC_CACHE is None:
        _NC_CACHE = _build_module()
    return _NC_CACHE


def _pos_emb_T():
    pos = np.arange(L, dtype=np.float32)[:, None]
    div = np.exp(np.arange(0, DM, 2, dtype=np.float32) * (-np.log(10000.0) / DM))
    pe = np.zeros((L, DM), np.float32)
    pe[:, 0::2] = np.sin(pos * div)
    pe[:, 1::2] = np.cos(pos * div)
    return pe.T.copy()


def _prep_inputs(inputs):
    import ml_dtypes
    f = lambda x: np.ascontiguousarray(np.asarray(x, dtype=np.float32))
    bf = lambda x: np.ascontiguousarray(x).astype(ml_dtypes.bfloat16)
    tokW = f(inputs["tok_conv_w"])                        # [DM, CIN, EK]
    inW = f(inputs["in_proj_w"])                          # [2DI, DM]
    cvw = f(inputs["dconv_w"])[:, 0, :]                   # [DI, DC]
    # convWT[(j,m), d] = in_proj_w[d, m] * dconv_w[d, j]
    convWT = (inW[:DI][None, :, :] * cvw.T[:, :, None]).transpose(0, 2, 1)  # [DC, DM, DI]
    convWT = np.ascontiguousarray(convWT.reshape(DC * DM, DI))
    attnb = f(inputs["attn_b"])
    brow = np.zeros((1, NCLS + NH), np.float32)
    brow[0, NCLS:] = attnb
    shared = {
        "tok_lhsT": bf(np.transpose(tokW, (2, 1, 0)).reshape(KD, DM)),
        "peT": _pos_emb_T(),
        "inWzT": bf(inW[DI:].T),
        "convWT": bf(convWT),
        "dcb": f(inputs["dconv_b"]).reshape(DI, 1),
        "xprojWT": bf(f(inputs["x_proj_w"]).T[:, list(range(DTR)) + list(range(DTR + DS, DTR + 2 * DS)) + list(range(DTR, DTR + DS))]),
        "dtWT": bf(f(inputs["dt_proj_w"]).T),
        "dtb": f(inputs["dt_proj_b"]).reshape(DI, 1),
        "Amat": (-np.exp(f(inputs["A_log"]))).astype(np.float32),
        "Dv": f(inputs["Dvec"]).reshape(DI, 1),
        "WoutT": bf(np.concatenate([f(inputs["out_proj_w"]).T,
                                    f(inputs["out_proj_w"]).T.sum(axis=1, keepdims=True)], axis=1)),
        "lng_bc": bf(np.broadcast_to(f(inputs["ln_g"]), (128, DM)).copy()),
        "lnb_bc": np.broadcast_to(f(inputs["ln_b"]), (128, DM)).copy(),
        "headWT": bf(np.concatenate([f(inputs["cls_w"]).T, f(inputs["attn_w"]).T], axis=1)),
        "biasrow": bf(brow),
        "onesrow": bf(np.ones((1, 128), np.float32)),
        "onec": np.ones((128, 1), np.float32),
        "epsc": np.full((128, 1), 1e-5, np.float32),
    }
    xTall = np.ascontiguousarray(f(inputs["x_enc"]).transpose(0, 2, 1))  # [B, CIN, L]
    xm = f(inputs["x_mark_enc"])
    per_core = []
    for c in range(NCORES):
        m = dict(shared)
        m["xT"] = np.ascontiguousarray(xTall[c * BLOC:(c + 1) * BLOC]).astype(ml_dtypes.bfloat16)
        m["xmark2"] = np.ascontiguousarray(xm[c * BLOC:(c + 1) * BLOC]).astype(ml_dtypes.bfloat16)
        per_core.append(m)
    return per_core


def kernel(**inputs) -> np.ndarray:
    from concourse.bass_utils import run_bass_kernel_spmd

    nc = _get_module()
    in_maps = _prep_inputs(inputs)
    res = run_bass_kernel_spmd(nc, in_maps, core_ids=list(range(NCORES)))
    return np.concatenate([res.results[c]["out"] for c in range(NCORES)], axis=0)


# revision 23
# speedup vs baseline: 1.4111x; 1.0914x over previous
"""Trainium2 Bass kernel for nn_Model_14998025797662 (Mamba-TimeVariant classifier).

Self-contained: hardcodes shapes. Data-parallel over batch: 16 samples ->
8 cores x 2 samples. Layout: channels-on-partitions, time-on-free.

v2: both per-core samples are packed along the free axis (T2 = 2048) so every
elementwise/scan op covers both samples in one instruction. The scan resets at
the sample boundary via a poisoned dt column (dt=+30 -> dA=exp(A*30)=0).
Engine budget: SSM scans pinned on DVE; dA exponentials on ACT; dBu/C-term
muls split DVE(bf16 2x)/Pool; the 60 state-accumulate adds run as SWDGE
accumulate-DMAs (free of engine time). B/C broadcasts via DRAM-row bounce.
Head transposes via xbar DMA-transpose. LayerNorm stats batched across the 16
time chunks to kill the serial scalar chain.
"""

import numpy as np

import concourse.bacc as bacc
import concourse.bass as bass
from concourse import mybir
from concourse.bass import ds, ts
from concourse.tile import TileContext

F32 = mybir.dt.float32
BF16 = mybir.dt.bfloat16
AF = mybir.ActivationFunctionType
OP = mybir.AluOpType
AX = mybir.AxisListType

B, L, CIN = 16, 1024, 12
DM, DS, DC, DI, DTR = 256, 16, 4, 512, 16
NCLS, NH, EK = 10, 8, 3
NCORES = 8
BLOC = B // NCORES          # 2 samples per core
T2 = BLOC * L               # 2048 combined free axis
NDT = DI // 128             # 4 d-tiles
NCH = T2 // 128             # 16 time chunks
KD = EK * CIN               # 36
PAD = DC - 1                # 3 pad cols per sample for the causal conv
EW = T2 + BLOC * PAD        # 2054 emb width


def _off(n):
    """emb col offset of 512-chunk n (pads at [0:3] and [1027:1030])."""
    return PAD + n * 512 + PAD * (n >= 2)


def _pool_dbu(s, d):
    # dBu feeds the DVE scan directly; keeping it on DVE avoids scan stalls
    return False


def _pool_term(s, d):
    # C-side terms are off the critical path (they feed SWDGE accum DMAs);
    # ~48 on Pool, 16 on DVE (SWDGE desc-gen also eats Pool time)
    if d == (s % 4) and s % 4 != 3:
        return False
    if s % 4 == 3 and d == ((s + 1) % 4):
        return False
    return True


def _patch_act_tables():
    """Bias ACT table-set selection so Exp and Ln resolve to the same set
    (avoids per-op table thrash). Idempotent."""
    import concourse.bacc as _bacc
    import concourse.hw_specs as _hw
    if getattr(_bacc, "_ant_act_tables_patched", False):
        return
    _orig = _hw.get_activation_tables

    def patched(arch):
        t = _orig(arch)
        both = None
        for name, fns in t.items():
            sn = {str(x).split(".")[-1] for x in fns}
            if "Exp" in sn and "Ln" in sn:
                both = name
                break
        if both is not None:
            for name, fns in t.items():
                if name == both:
                    continue
                fns.discard(mybir.ActivationFunctionType.Exp)
                fns.discard(mybir.ActivationFunctionType.Ln)
        return t

    _bacc.get_activation_tables = patched
    _bacc._ant_act_tables_patched = True


def _build_module():
    _patch_act_tables()
    nc = bacc.Bacc("TRN2", target_bir_lowering=False)

    def din(name, shape, dt=F32):
        return nc.dram_tensor(name, shape, dt, kind="ExternalInput")

    xT = din("xT", [BLOC, CIN, L], BF16)
    xmark2 = din("xmark2", [BLOC, L], BF16)
    tok_lhsT = din("tok_lhsT", [KD, DM], BF16)
    peT = din("peT", [DM, L])
    inWzT = din("inWzT", [DM, DI], BF16)    # z half of in_proj
    convWT = din("convWT", [DC * DM, DI], BF16)
    dcb = din("dcb", [DI, 1])
    xprojWT = din("xprojWT", [DI, DTR + 2 * DS], BF16)
    dtWT = din("dtWT", [DTR, DI], BF16)
    dtb = din("dtb", [DI, 1])
    Amat = din("Amat", [DI, DS])
    Dv = din("Dv", [DI, 1])
    WoutT = din("WoutT", [DI, DM + 1], BF16)
    lng_bc = din("lng_bc", [128, DM], BF16)
    lnb_bc = din("lnb_bc", [128, DM])
    headWT = din("headWT", [DM, NCLS + NH], BF16)
    biasrow = din("biasrow", [1, NCLS + NH], BF16)
    onesrow = din("onesrow", [1, 128], BF16)
    onec = din("onec", [128, 1])
    epsc = din("epsc", [128, 1])

    out = nc.dram_tensor("out", [BLOC, NCLS], F32, kind="ExternalOutput")
    scr_bc = nc.dram_tensor("scr_bc", [2 * DS, T2], BF16)   # rows 0:16 C, 16:32 B

    with TileContext(nc) as tc:
        with (
            tc.tile_pool(name="const", bufs=1) as cp,
            tc.tile_pool(name="persist", bufs=1) as pp,
            tc.tile_pool(name="work", bufs=2) as wp,
            tc.tile_pool(name="small", bufs=2) as sp,
            tc.tile_pool(name="psumr", bufs=3, space="PSUM") as psr,
        ):
            def cload(name, shape, src, dt=F32):
                t = cp.tile(shape, dt, name=f"c_{name}")
                nc.sync.dma_start(t[:], src)
                return t

            # inputs + first-use weights load first so stage A starts early
            tokW_sb = cload("tokW", [KD, DM], tok_lhsT[:], BF16)
            rhs36 = pp.tile([KD, T2], BF16, name="rhs36", tag="skinny", bufs=2)
            for b in range(BLOC):
                c0 = b * L
                nc.scalar.dma_start(rhs36[12:24, c0:c0 + L], xT[b, :, :])
                nc.scalar.dma_start(rhs36[0:12, c0 + 1:c0 + L], xT[b, :, 0:L - 1])
                nc.scalar.dma_start(rhs36[0:12, c0:c0 + 1], xT[b, :, 0:1])
                nc.scalar.dma_start(rhs36[24:36, c0:c0 + L - 1], xT[b, :, 1:L])
                nc.scalar.dma_start(rhs36[24:36, c0 + L - 1:c0 + L], xT[b, :, L - 1:L])
            pe_sb = []
            for m in range(2):
                # staged in the dbu work-tag ring (dead before stage B uses it)
                pt = wp.tile([128, L], F32, name=f"pe{m}", tag="dbu", bufs=2)
                nc.sync.dma_start(pt[:], peT[ts(m, 128), :])
                pe_sb.append(pt)
            inWz_sb = [cload(f"inWz{k}", [128, DI], inWzT[ts(k, 128), :], BF16) for k in range(2)]
            convW_sb = [cload(f"cvW{k}", [128, DI], convWT[ts(k, 128), :], BF16) for k in range(8)]
            dcb_sb = [cload(f"dcb{d}", [128, 1], dcb[ts(d, 128), :]) for d in range(NDT)]
            xprojW_sb = [cload(f"xpW{d}", [128, DTR + 2 * DS], xprojWT[ts(d, 128), :], BF16) for d in range(NDT)]
            dtW_sb = cload("dtW", [DTR, DI], dtWT[:], BF16)
            dtb_sb = [cload(f"dtb{d}", [128, 1], dtb[ts(d, 128), :]) for d in range(NDT)]
            A_sb = [cload(f"A{d}", [128, DS], Amat[ts(d, 128), :]) for d in range(NDT)]
            Dv_sb = [cload(f"Dv{d}", [128, 1], Dv[ts(d, 128), :]) for d in range(NDT)]
            Wout_sb = [cload(f"Wo{d}", [128, DM + 1], WoutT[ts(d, 128), :], BF16) for d in range(NDT)]
            lng_sb = cload("lng", [128, DM], lng_bc[:], BF16)
            lnb_sb = cload("lnb", [128, DM], lnb_bc[:])
            headW_sb = [cload(f"hW{k}", [128, NCLS + NH], headWT[ts(k, 128), :], BF16) for k in range(2)]
            bias_sb = cload("biasrow", [1, NCLS + NH], biasrow[:], BF16)
            ones_sb = cload("onesrow", [1, 128], onesrow[:], BF16)
            one_sb = cload("onec", [128, 1], onec[:])
            eps_sb = cload("epsc", [128, 1], epsc[:])
            xmrow = wp.tile([BLOC, L], BF16, name="xmrow", tag="xmk", bufs=1)
            nc.sync.dma_start(xmrow[:], xmark2[:, :])

            # ======== stage A ========

            emb_sb = [pp.tile([128, EW], BF16, name=f"emb{m}", tag="embt", bufs=2) for m in range(2)]
            for m in range(2):
                nc.vector.memset(emb_sb[m][:, 0:PAD], 0.0)
                nc.vector.memset(emb_sb[m][:, PAD + L:PAD + L + PAD], 0.0)

            sz_sb = [pp.tile([128, T2], BF16, name=f"sz{d}", tag=f"sz{d}") for d in range(NDT)]
            u0_sb = [pp.tile([128, T2], BF16, name=f"u0{d}", tag=f"u0{d}") for d in range(NDT)]
            xdbl_sb = pp.tile([DTR + 2 * DS, T2], BF16, name="xdbl", tag="skinny", bufs=2)
            dt_sb = [pp.tile([128, T2], BF16, name=f"dt{d}", tag=f"dt{d}") for d in range(NDT)]

            # pass 1 (silu act-table): emb, z-silu, conv-silu per chunk
            for n in range(4):
                o = _off(n)
                # emb chunk: tok conv + positional embedding
                for m in range(2):
                    ps = psr.tile([128, 512], F32, name=f"eps{m}{n}", tag="ps512")
                    nc.tensor.matmul(ps[:], tokW_sb[:, ts(m, 128)], rhs36[:, ts(n, 512)],
                                     start=True, stop=True)
                    nc.vector.tensor_add(emb_sb[m][:, ds(o, 512)], ps[:],
                                         pe_sb[m][:, ds((n % 2) * 512, 512)])
                # z half -> silu
                for d in range(NDT):
                    ps = psr.tile([128, 512], F32, name=f"z{d}{n}", tag="ps512")
                    for k in range(2):
                        nc.tensor.matmul(ps[:], inWz_sb[k][:, ts(d, 128)],
                                         emb_sb[k][:, ds(o, 512)],
                                         start=(k == 0), stop=(k == 1))
                    nc.scalar.activation(sz_sb[d][:, ts(n, 512)], ps[:], AF.Silu)
                # fused causal conv of in_proj x-half -> silu
                for d in range(NDT):
                    ps = psr.tile([128, 512], F32, name=f"u{d}{n}", tag="ps512")
                    for k in range(8):
                        j = k // 2
                        nc.tensor.matmul(ps[:], convW_sb[k][:, ts(d, 128)],
                                         emb_sb[k % 2][:, ds(o - PAD + j, 512)],
                                         start=(k == 0), stop=(k == 7))
                    nc.scalar.activation(u0_sb[d][:, ts(n, 512)], ps[:], AF.Silu,
                                         bias=dcb_sb[d][:, 0:1])
            # pass 2 (no act table): x_proj -> x_dblT
            for n in range(4):
                ps = psr.tile([DTR + 2 * DS, 512], F32, name=f"xd{n}", tag="ps512")
                for k in range(NDT):
                    nc.tensor.matmul(ps[:], xprojW_sb[k][:], u0_sb[k][:, ts(n, 512)],
                                     start=(k == 0), stop=(k == NDT - 1))
                nc.vector.tensor_copy(xdbl_sb[:, ts(n, 512)], ps[:])
            # stage B/C broadcast rows to DRAM (single wide write)
            nc.sync.dma_start(scr_bc[:, :], xdbl_sb[DTR:DTR + 2 * DS, :])

            # pass 3 (exp/ln act-table): dt = softplus, d-outer so stage B can
            # start on d=0 while later d-tiles still compute. w = dt*u, then
            # poison the dt col at the sample boundary so dA goes to 0 there
            # (scan state reset).
            wT_sb = [pp.tile([128, T2], BF16, name=f"w{d}", tag=f"w{d}") for d in range(NDT)]
            for d in range(NDT):
                for n in range(4):
                    ps = psr.tile([128, 512], F32, name=f"dtp{d}{n}", tag="ps512")
                    nc.tensor.matmul(ps[:], dtW_sb[:, ts(d, 128)], xdbl_sb[0:DTR, ts(n, 512)],
                                     start=True, stop=True)
                    esp = psr.tile([128, 512], F32, name=f"esp{d}{n}", tag="ps512")
                    nc.scalar.activation(esp[:], ps[:], AF.Exp, bias=dtb_sb[d][:, 0:1])
                    nc.scalar.activation(dt_sb[d][:, ts(n, 512)], esp[:], AF.Ln,
                                         bias=one_sb[:, 0:1])
                nc.vector.tensor_mul(wT_sb[d][:], dt_sb[d][:], u0_sb[d][:])
                nc.vector.memset(dt_sb[d][:, L:L + 1], 30.0)

            # ======== stage B: 16 SSM states ========
            # acc/term tiles pair adjacent d-tiles along free so one SWDGE
            # accumulate-DMA covers two d-tiles (halves Pool desc-gen time)
            accp = [pp.tile([128, 2 * T2], BF16, name=f"accp{p}", tag=f"accp{p}") for p in range(2)]
            acc = [accp[d // 2][:, ds((d % 2) * T2, T2)] for d in range(NDT)]
            for s in range(DS):
                bbc = wp.tile([128, T2], BF16, name=f"bbc{s}", tag="bbc", bufs=2)
                nc.sync.dma_start(bbc[:], scr_bc[DS + s:DS + s + 1, :].to_broadcast((128, T2)))
                cbc = wp.tile([128, T2], BF16, name=f"cbc{s}", tag="cbc", bufs=2)
                nc.sync.dma_start(cbc[:], scr_bc[s:s + 1, :].to_broadcast((128, T2)))
                dAs, dBus, hs = [], [], []
                for d in range(NDT):
                    dA = wp.tile([128, T2], F32, name=f"dA{s}{d}", tag="dA", bufs=2)
                    nc.scalar.activation(dA[:], dt_sb[d][:], AF.Exp, scale=A_sb[d][:, s:s + 1])
                    dAs.append(dA)
                for d in range(NDT):
                    dBu = wp.tile([128, T2], BF16, name=f"dBu{s}{d}", tag="dbu", bufs=2)
                    eng = nc.gpsimd if _pool_dbu(s, d) else nc.vector
                    eng.tensor_mul(dBu[:], wT_sb[d][:], bbc[:])
                    dBus.append(dBu)
                for d in range(NDT):
                    h = wp.tile([128, T2], BF16, name=f"h{s}{d}", tag="hh", bufs=4)
                    nc.vector.tensor_tensor_scan(h[:], dAs[d][:], dBus[d][:], 0.0,
                                                 op0=OP.mult, op1=OP.add)
                    hs.append(h)
                if s == 0:
                    for d in range(NDT):
                        eng = nc.gpsimd if _pool_term(s, d) else nc.vector
                        eng.tensor_mul(acc[d], hs[d][:], cbc[:])
                else:
                    for p in range(2):
                        tp2 = wp.tile([128, 2 * T2], BF16, name=f"term{s}{p}", tag="term", bufs=3)
                        for i in range(2):
                            d = 2 * p + i
                            eng = nc.gpsimd if _pool_term(s, d) else nc.vector
                            eng.tensor_mul(tp2[:, ds(i * T2, T2)], hs[d][:], cbc[:])
                        nc.gpsimd.dma_start(accp[p][:], tp2[:], accum_op=OP.add)

            # ytot = (acc + u*D) * sz, stored back into the w tiles
            for d in range(NDT):
                t1 = wp.tile([128, T2], BF16, name=f"yt1{d}", tag="dbu", bufs=2)
                nc.vector.scalar_tensor_tensor(t1[:], u0_sb[d][:], Dv_sb[d][:, 0:1],
                                               acc[d], op0=OP.mult, op1=OP.add)
                nc.vector.tensor_mul(wT_sb[d][:], t1[:], sz_sb[d][:])

            # ======== stage C ========
            ssum_all = sp.tile([128, NCH], F32, name="ssum_all", tag="ssum", bufs=1)
            sqs_all = sp.tile([128, NCH], F32, name="sqs_all", tag="sqs", bufs=1)
            mos_half = [pp.tile([128, T2], BF16, name=f"mosh{i}", tag="skinny", bufs=2)
                        for i in range(2)]
            mosb = [mos_half[t // 8][:, ds(256 * (t % 8), DM)] for t in range(NCH)]
            for t in range(NCH):
                mp = psr.tile([128, DM + 1], F32, name=f"mo{t}", tag="MO", bufs=3)
                for d in range(NDT):
                    nc.tensor.matmul(mp[:], wT_sb[d][:, ts(t, 128)], Wout_sb[d][:],
                                     start=(d == 0), stop=(d == NDT - 1))
                nc.scalar.copy(mosb[t], mp[:, 0:DM])
                nc.vector.tensor_copy(ssum_all[:, t:t + 1], mp[:, DM:DM + 1])
                sq = wp.tile([128, DM], BF16, name=f"sq{t}", tag="xn", bufs=3)
                nc.vector.scalar_tensor_tensor(sq[:], mosb[t], 1.0, mosb[t],
                                               op0=OP.mult, op1=OP.mult,
                                               accum_out=sqs_all[:, t:t + 1])

            # batched LN stats over all 16 chunks
            mun = sp.tile([128, NCH], F32, name="mun", tag="mun")
            nc.vector.tensor_scalar_mul(mun[:], ssum_all[:], -1.0 / DM)
            m2t = sp.tile([128, NCH], F32, name="m2t", tag="m2t")
            nc.vector.tensor_scalar_mul(m2t[:], sqs_all[:], 1.0 / DM)
            msq = sp.tile([128, NCH], F32, name="msq", tag="msq")
            nc.vector.tensor_mul(msq[:], mun[:], mun[:])
            var = sp.tile([128, NCH], F32, name="var", tag="var")
            nc.vector.tensor_sub(var[:], m2t[:], msq[:])
            lnv = sp.tile([128, NCH], F32, name="lnv", tag="lnv")
            nc.scalar.activation(lnv[:], var[:], AF.Ln, bias=eps_sb[:, 0:1])
            rstd = sp.tile([128, NCH], F32, name="rstd", tag="rstd")
            nc.scalar.activation(rstd[:], lnv[:], AF.Exp, scale=-0.5)
            nmr = sp.tile([128, NCH], F32, name="nmr", tag="nmr")
            nc.vector.tensor_mul(nmr[:], mun[:], rstd[:])

            row = wp.tile([BLOC, L], F32, name="row_am", tag="term", bufs=3)
            lg_all = []
            for t in range(NCH):
                xn = wp.tile([128, DM], BF16, name=f"xn{t}", tag="xn", bufs=3)
                nc.scalar.activation(xn[:], mosb[t], AF.Identity,
                                     bias=nmr[:, t:t + 1], scale=rstd[:, t:t + 1])
                t1 = wp.tile([128, DM], BF16, name=f"t1{t}", tag="t1", bufs=3)
                nc.vector.tensor_mul(t1[:], xn[:], lng_sb[:])
                t2 = wp.tile([128, DM], BF16, name=f"t2{t}", tag="t2", bufs=3)
                nc.gpsimd.tensor_add(t2[:], t1[:], lnb_sb[:])
                mam = wp.tile([128, DM], BF16, name=f"mam{t}", tag="mam", bufs=4)
                nc.scalar.activation(mam[:], t2[:], AF.Silu)
                moT = [wp.tile([128, 128], BF16, name=f"moT{t}{m}", tag="moT", bufs=6)
                       for m in range(2)]
                nc.sync.dma_start(moT[0][:], mam[:, 0:128], transpose=True)
                nc.scalar.dma_start(moT[1][:], mam[:, 128:256], transpose=True)
                hd = psr.tile([128, NCLS + NH], F32, name=f"hd{t}", tag="HD", bufs=2)
                for k in range(2):
                    nc.tensor.matmul(hd[:], moT[k][:], headW_sb[k][:],
                                     start=(k == 0), stop=False)
                nc.tensor.matmul(hd[:], ones_sb[:], bias_sb[:], start=False, stop=True)
                lg = pp.tile([128, NCLS], F32, name=f"lg{t}", tag=f"lg{t}")
                nc.vector.tensor_copy(lg[:], hd[:, 0:NCLS])
                lg_all.append(lg)
                am = sp.tile([128, 1], F32, name=f"am{t}", tag="am", bufs=2)
                nc.vector.reduce_max(am[:], hd[:, NCLS:NCLS + NH], axis=AX.X)
                q = nc.sync if t % 2 == 0 else nc.scalar
                q.dma_start(row[t // 8:t // 8 + 1, ds(128 * (t % 8), 128)], am[:])

            # tail: per-sample softmax over time (sample = partition)
            mx = sp.tile([BLOC, 1], F32, name="mx", tag="mx")
            nc.vector.reduce_max(mx[:], row[:], axis=AX.X)
            nmx = sp.tile([BLOC, 1], F32, name="nmx", tag="nmx")
            nc.vector.tensor_scalar_mul(nmx[:], mx[:], -1.0)
            ex = wp.tile([BLOC, L], F32, name="ex", tag="term", bufs=3)
            esum = sp.tile([BLOC, 1], F32, name="esum", tag="esum")
            nc.scalar.activation(ex[:], row[:], AF.Exp, bias=nmx[:, 0:1],
                                 accum_out=esum[:])
            rec = sp.tile([BLOC, 1], F32, name="rec", tag="rec")
            nc.vector.reciprocal(rec[:], esum[:])
            wx = wp.tile([BLOC, L], F32, name="wx", tag="term", bufs=3)
            nc.vector.scalar_tensor_tensor(wx[:], ex[:], rec[:, 0:1], xmrow[:],
                                           op0=OP.mult, op1=OP.mult)
            wxc = []
            for t in range(NCH):
                wc = sp.tile([128, 1], F32, name=f"wxc{t}", tag=f"wxc{t}")
                q = nc.sync if t % 2 == 0 else nc.scalar
                q.dma_start(wc[:], wx[t // 8:t // 8 + 1, ds(128 * (t % 8), 128)])
                wxc.append(wc)
            for b in range(BLOC):
                ops = psr.tile([NCLS, 1], F32, name=f"ops{b}", tag="HD", bufs=2)
                for i in range(8):
                    t = 8 * b + i
                    nc.tensor.matmul(ops[:], lg_all[t][:], wxc[t][:],
                                     start=(i == 0), stop=(i == 7))
                oc = sp.tile([NCLS, 1], F32, name=f"oc{b}", tag=f"oc{b}")
                nc.vector.tensor_copy(oc[:], ops[:])
                nc.sync.dma_start(out[b, :], oc[:])

    nc.finalize()
    return nc


_NC_CACHE = None


def _get_module():
    global _NC_CACHE
    if _NC_CACHE is None:
        _NC_CACHE = _build_module()
    return _NC_CACHE


def _pos_emb_T():
    pos = np.arange(L, dtype=np.float32)[:, None]
    div = np.exp(np.arange(0, DM, 2, dtype=np.float32) * (-np.log(10000.0) / DM))
    pe = np.zeros((L, DM), np.float32)
    pe[:, 0::2] = np.sin(pos * div)
    pe[:, 1::2] = np.cos(pos * div)
    return pe.T.copy()


def _prep_inputs(inputs):
    import ml_dtypes
    f = lambda x: np.ascontiguousarray(np.asarray(x, dtype=np.float32))
    bf = lambda x: np.ascontiguousarray(x).astype(ml_dtypes.bfloat16)
    tokW = f(inputs["tok_conv_w"])                        # [DM, CIN, EK]
    inW = f(inputs["in_proj_w"])                          # [2DI, DM]
    cvw = f(inputs["dconv_w"])[:, 0, :]                   # [DI, DC]
    # convWT[(j,m), d] = in_proj_w[d, m] * dconv_w[d, j]
    convWT = (inW[:DI][None, :, :] * cvw.T[:, :, None]).transpose(0, 2, 1)  # [DC, DM, DI]
    convWT = np.ascontiguousarray(convWT.reshape(DC * DM, DI))
    attnb = f(inputs["attn_b"])
    brow = np.zeros((1, NCLS + NH), np.float32)
    brow[0, NCLS:] = attnb
    shared = {
        "tok_lhsT": bf(np.transpose(tokW, (2, 1, 0)).reshape(KD, DM)),
        "peT": _pos_emb_T(),
        "inWzT": bf(inW[DI:].T),
        "convWT": bf(convWT),
        "dcb": f(inputs["dconv_b"]).reshape(DI, 1),
        "xprojWT": bf(f(inputs["x_proj_w"]).T[:, list(range(DTR)) + list(range(DTR + DS, DTR + 2 * DS)) + list(range(DTR, DTR + DS))]),
        "dtWT": bf(f(inputs["dt_proj_w"]).T),
        "dtb": f(inputs["dt_proj_b"]).reshape(DI, 1),
        "Amat": (-np.exp(f(inputs["A_log"]))).astype(np.float32),
        "Dv": f(inputs["Dvec"]).reshape(DI, 1),
        "WoutT": bf(np.concatenate([f(inputs["out_proj_w"]).T,
                                    f(inputs["out_proj_w"]).T.sum(axis=1, keepdims=True)], axis=1)),
        "lng_bc": bf(np.broadcast_to(f(inputs["ln_g"]), (128, DM)).copy()),
        "lnb_bc": np.broadcast_to(f(inputs["ln_b"]), (128, DM)).copy(),
        "headWT": bf(np.concatenate([f(inputs["cls_w"]).T, f(inputs["attn_w"]).T], axis=1)),
        "biasrow": bf(brow),
        "onesrow": bf(np.ones((1, 128), np.float32)),
        "onec": np.ones((128, 1), np.float32),
        "epsc": np.full((128, 1), 1e-5, np.float32),
    }
    xTall = np.ascontiguousarray(f(inputs["x_enc"]).transpose(0, 2, 1))  # [B, CIN, L]
    xm = f(inputs["x_mark_enc"])
    per_core = []
    for c in range(NCORES):
        m = dict(shared)
        m["xT"] = np.ascontiguousarray(xTall[c * BLOC:(c + 1) * BLOC]).astype(ml_dtypes.bfloat16)
        m["xmark2"] = np.ascontiguousarray(xm[c * BLOC:(c + 1) * BLOC]).astype(ml_dtypes.bfloat16)
        per_core.append(m)
    return per_core


def kernel(**inputs) -> np.ndarray:
    from concourse.bass_utils import run_bass_kernel_spmd

    nc = _get_module()
    in_maps = _prep_inputs(inputs)
    res = run_bass_kernel_spmd(nc, in_maps, core_ids=list(range(NCORES)))
    return np.concatenate([res.results[c]["out"] for c in range(NCORES)], axis=0)
